# revision 6
# baseline (speedup 1.0000x reference)
"""Trainium2 Bass kernel for the additive-attention transformer.

Sharding: 8 cores = (batch b in 0..3) x (sequence half in 0..1); each core
owns 128 query rows through 3 encoder layers; core pairs AllGather updated
halves after layers 1, 2.  The tiny layer-4 attention and the head run on
the host in fp32.

Scores trick: tanh(q+k) is approximated by an 11-term sine series
tanh(x) ~ sum_m b_m sin(m*w0*x) fitted on [-6.3, 6.3] (max err 5.5e-4).
With the phase-split identity
    sin(A+B) = sin(A+pi/4) sin(B+pi/4) - sin(A-pi/4) sin(B-pi/4)
the [S,S,H] additive-attention tensor never materializes: scores become 22
PE matmul chunks contracting over H between per-side sine features.  ACT's
sin table only covers ~[-3.4, 3.4], so harmonic angles are range-reduced
with batched add_range_wrap ops on DVE.  Softmax exp is an fp16 Horner
polynomial on DVE (keeps ACT on the single trig table set all kernel).
"""

import numpy as np
import ml_dtypes

import concourse.bass as bass
import concourse.mybir as mybir
import concourse.tile as tile
from concourse import bacc
from concourse.bass_utils import run_bass_kernel_spmd
from concourse.masks import make_identity

F32 = mybir.dt.float32
F16 = mybir.dt.float16
BF16 = mybir.dt.bfloat16
AF = mybir.ActivationFunctionType
ALU = mybir.AluOpType

V, H, B, S = 1280, 128, 4, 256
P = 128
VC = V // P      # 10 v-chunks
NCORES = 8
EPS = 1e-5

M = 11                       # sine harmonics
W0 = float(np.pi / 8.0)      # base frequency (L = 8)
PI = float(np.pi)

# tanh(x) ~ sum b[m-1] sin(m*W0*x), minimax fit on [-6.3, 6.3]
BCO = [1.221065310324293, -0.042075320460250754, 0.3010164469565419,
       -0.04480331164706743, 0.10878841500856154, -0.027212423716920286,
       0.039297859521858644, -0.011227877114154771, 0.012187029191205184,
       -0.00275684427587024, 0.0026936173814147953]
# exp(x) ~ 1 + x(a0 + x(a1 + ... )) minimax on [-1.35, 1.35]
ACO = [1.0000938327828903, 0.4999331310633901, 0.16617314063173236,
       0.04175343105133384, 0.008934435304589107, 0.001402258553964398]

_CACHE = {}


def _build():
    nc = bacc.Bacc("TRN2", target_bir_lowering=False, debug=False,
                   num_devices=NCORES)

    # ---- I/O ----
    xo32_in = nc.dram_tensor("xo32", [P, V], F32, kind="ExternalInput")
    xqt_in = nc.dram_tensor("xqt", [P, VC, P], BF16, kind="ExternalInput")
    xf_in = nc.dram_tensor("xf", [P, 2, V], BF16, kind="ExternalInput")
    xft_in = nc.dram_tensor("xft", [P, VC, S], BF16, kind="ExternalInput")
    w_in = {}
    for l in range(3):
        w_in[f"wq{l}"] = nc.dram_tensor(f"wq{l}", [P, VC, H], BF16, kind="ExternalInput")
        w_in[f"wk{l}"] = nc.dram_tensor(f"wk{l}", [P, VC, H], BF16, kind="ExternalInput")
        w_in[f"wvbp{l}"] = nc.dram_tensor(f"wvbp{l}", [P, M], F32, kind="ExternalInput")
        w_in[f"wvbm{l}"] = nc.dram_tensor(f"wvbm{l}", [P, M], F32, kind="ExternalInput")
        w_in[f"w1{l}"] = nc.dram_tensor(f"w1{l}", [P, VC, H], BF16, kind="ExternalInput")
        w_in[f"b1{l}"] = nc.dram_tensor(f"b1{l}", [P, 1], F32, kind="ExternalInput")
        w_in[f"w2{l}"] = nc.dram_tensor(f"w2{l}", [P, V], BF16, kind="ExternalInput")
        w_in[f"b2{l}"] = nc.dram_tensor(f"b2{l}", [1, V], BF16, kind="ExternalInput")
    zout = nc.dram_tensor("zout", [P, V], F32, kind="ExternalOutput")

    agin = [nc.dram_tensor(f"agin{l}", [P, 2 * V], BF16) for l in range(2)]
    agout = [nc.dram_tensor(f"agout{l}", [2, P, 2 * V], BF16) for l in range(2)]
    groups = [[0, 1], [2, 3], [4, 5], [6, 7]]

    with tile.TileContext(nc) as tc:
        with tc.tile_pool(name="persist", bufs=1) as pp, \
             tc.tile_pool(name="xbuf", bufs=2) as xb, \
             tc.tile_pool(name="scratch", bufs=2) as sc, \
             tc.tile_pool(name="feat", bufs=1) as fp_pool, \
             tc.tile_pool(name="ps", bufs=1, space="PSUM") as ps, \
             tc.tile_pool(name="ps2", bufs=1, space="PSUM") as ps2:

            ident = pp.tile([P, P], BF16, tag="ident")
            make_identity(nc, ident[:])
            ones = pp.tile([P, 1], BF16, tag="ones")
            nc.vector.memset(ones[:], 1.0)
            onesrow = pp.tile([1, P], BF16, tag="onesrow")
            nc.vector.memset(onesrow[:], 1.0)
            biasp = pp.tile([P, 1], F32, tag="biasp")
            nc.vector.memset(biasp[:], PI / 4.0)
            biasm = pp.tile([P, 1], F32, tag="biasm")
            nc.vector.memset(biasm[:], -PI / 4.0)

            # initial X buffers first (critical path), then weights by layer
            xo32 = xb.tile([P, V], F32, tag="xo32")
            nc.sync.dma_start(xo32[:], xo32_in[:, :])
            xqt = xb.tile([P, VC, P], BF16, tag="xqt")
            nc.sync.dma_start(xqt[:], xqt_in[:, :, :])
            xf = xb.tile([P, 2, V], BF16, tag="xf")
            nc.sync.dma_start(xf[:], xf_in[:, :, :])
            xft = xb.tile([P, VC, S], BF16, tag="xft")
            nc.sync.dma_start(xft[:], xft_in[:, :, :])

            w = {}
            for l in range(3):
                for pre in ("wq", "wk", "wvbp", "wvbm", "w1", "b1", "w2", "b2"):
                    k = f"{pre}{l}"
                    t = w_in[k]
                    tl = pp.tile(list(t.shape), t.dtype, tag=k)
                    nc.sync.dma_start(tl[:], t[(slice(None),) * len(t.shape)])
                    w[k] = tl

            for l in range(3):
                # ---- projections: qkt = [qT own | kT abs] in one PSUM tile
                qkt = ps.tile([P, 3 * P], F32, tag="qkt")
                for c in range(VC):
                    nc.tensor.matmul(qkt[:, 0:P], w[f"wq{l}"][:, c, :],
                                     xqt[:, c, :],
                                     start=(c == 0), stop=(c == VC - 1))
                for c in range(VC):
                    nc.tensor.matmul(qkt[:, P:3 * P], w[f"wk{l}"][:, c, :],
                                     xft[:, c, :],
                                     start=(c == 0), stop=(c == VC - 1))

                # ---- qkt to SBUF (gpsimd cannot read PSUM)
                qkt32 = sc.tile([P, 3 * P], F32, tag="qkt32")
                nc.vector.tensor_copy(qkt32[:], qkt[:])

                # ---- harmonic angles m=2..11 (idx 0..9), fp32
                th = fp_pool.tile([P, M - 1, 3 * P], F32, tag="th",
                                  name=f"th_{l}")
                thw = fp_pool.tile([P, M - 1, 3 * P], F32, tag="thw",
                                   name=f"thw_{l}")
                for mi in range(M - 1):
                    m = mi + 2
                    eng = nc.gpsimd if m <= 6 else nc.vector
                    eng.tensor_scalar(out=th[:, mi, :], in0=qkt32[:],
                                      scalar1=float(m * W0), scalar2=None,
                                      op0=ALU.mult)
                # wrap into [-pi, pi]: m<=5 one wrap, m>=6 two wraps
                nc.vector.add_range_wrap(out=thw[:, 0:4, :], in_=th[:, 0:4, :],
                                         shift=0.0, bound=PI, period=2 * PI)
                nc.vector.add_range_wrap(out=thw[:, 4:10, :], in_=th[:, 4:10, :],
                                         shift=0.0, bound=3 * PI, period=4 * PI)
                nc.vector.add_range_wrap(out=thw[:, 4:10, :], in_=thw[:, 4:10, :],
                                         shift=0.0, bound=PI, period=2 * PI)

                # ---- features sin(theta +- pi/4), bf16
                fpl = fp_pool.tile([P, M, 3 * P], BF16, tag="fpl",
                                   name=f"fpl_{l}")
                fml = fp_pool.tile([P, M, 3 * P], BF16, tag="fml",
                                   name=f"fml_{l}")
                nc.scalar.activation(out=fpl[:, 0, :], in_=qkt32[:], func=AF.Sin,
                                     scale=W0, bias=biasp[:])
                nc.scalar.activation(out=fml[:, 0, :], in_=qkt32[:], func=AF.Sin,
                                     scale=W0, bias=biasm[:])
                nc.scalar.activation(out=fpl[:, 1:M, :], in_=thw[:], func=AF.Sin,
                                     bias=biasp[:])
                nc.scalar.activation(out=fml[:, 1:M, :], in_=thw[:], func=AF.Sin,
                                     bias=biasm[:])

                # ---- q-side scaling by +-b_m * wv
                qfp = fp_pool.tile([P, M, P], BF16, tag="qfp", name=f"qfp_{l}")
                qfm = fp_pool.tile([P, M, P], BF16, tag="qfm", name=f"qfm_{l}")
                for mi in range(M):
                    nc.vector.tensor_scalar(
                        out=qfp[:, mi, :], in0=fpl[:, mi, 0:P],
                        scalar1=w[f"wvbp{l}"][:, mi:mi + 1], scalar2=None,
                        op0=ALU.mult)
                    nc.vector.tensor_scalar(
                        out=qfm[:, mi, :], in0=fml[:, mi, 0:P],
                        scalar1=w[f"wvbm{l}"][:, mi:mi + 1], scalar2=None,
                        op0=ALU.mult)

                # ---- scores^T [j(half h), i] accumulated over 22 chunks
                sct = ps.tile([P, S], F32, tag="sct")
                for h in range(2):
                    for mi in range(M):
                        nc.tensor.matmul(sct[:, h * P:(h + 1) * P],
                                         fpl[:, mi, (1 + h) * P:(2 + h) * P],
                                         qfp[:, mi, :],
                                         start=(mi == 0), stop=False)
                    for mi in range(M):
                        nc.tensor.matmul(sct[:, h * P:(h + 1) * P],
                                         fml[:, mi, (1 + h) * P:(2 + h) * P],
                                         qfm[:, mi, :],
                                         start=False, stop=(mi == M - 1))

                # ---- softmax exp via fp16 Horner on DVE
                sc16 = sc.tile([P, S], F16, tag="sc16")
                nc.vector.tensor_copy(sc16[:], sct[:])
                pch = sc.tile([P, S], F16, tag="pch")
                nc.vector.tensor_scalar(out=pch[:], in0=sc16[:],
                                        scalar1=float(ACO[5]), scalar2=None,
                                        op0=ALU.mult)
                for k in (4, 3, 2, 1, 0):
                    nc.vector.scalar_tensor_tensor(
                        out=pch[:], in0=pch[:], scalar=float(ACO[k]),
                        in1=sc16[:], op0=ALU.add, op1=ALU.mult)
                expt = sc.tile([P, S], BF16, tag="expt")
                nc.vector.tensor_scalar(out=expt[:], in0=pch[:], scalar1=1.0,
                                        scalar2=None, op0=ALU.add)

                # ---- sums + attnV
                sums = ps.tile([P, 1], F32, tag="sct")
                for h in range(2):
                    nc.tensor.matmul(sums[:], expt[:, h * P:(h + 1) * P],
                                     ones[:], start=(h == 0), stop=(h == 1))
                rin = sc.tile([P, 1], F32, tag="rin")
                nc.vector.reciprocal(rin[:], sums[:])

                av = ps.tile([P, V], F32, tag="big")
                for off in range(0, V, 512):
                    n = min(512, V - off)
                    for h in range(2):
                        nc.tensor.matmul(av[:, off:off + n],
                                         expt[:, h * P:(h + 1) * P],
                                         xf[:, h, off:off + n],
                                         start=(h == 0), stop=(h == 1))

                # ---- ax = av/sums + X, LN stats
                ax = sc.tile([P, V], F32, tag="ax")
                nc.vector.scalar_tensor_tensor(out=ax[:], in0=av[:],
                                               scalar=rin[:], in1=xo32[:],
                                               op0=ALU.mult, op1=ALU.add)
                stats = sc.tile([P, 5, 6], F32, tag="stats")
                axg = ax[:].rearrange("p (n s) -> p n s", s=256)
                for g in range(5):
                    nc.vector.bn_stats(out=stats[:, g, :], in_=axg[:, g, :])
                mv = sc.tile([P, 2], F32, tag="mv")
                nc.vector.bn_aggr(out=mv[:], in_=stats[:])
                vv = sc.tile([P, 1], F32, tag="vv")
                nc.vector.tensor_scalar(out=vv[:], in0=mv[:, 1:2], scalar1=EPS,
                                        scalar2=None, op0=ALU.add)
                s_ = sc.tile([P, 1], F32, tag="s_")
                nc.vector.reciprocal(s_[:], vv[:])
                r_ = sc.tile([P, 1], F32, tag="r_")
                nc.vector.tensor_scalar(out=r_[:], in0=s_[:], scalar1=0.4315,
                                        scalar2=0.361, op0=ALU.mult, op1=ALU.add)
                t1 = sc.tile([P, 1], F32, tag="t1")
                for _ in range(3):
                    nc.vector.tensor_mul(out=t1[:], in0=vv[:], in1=r_[:])
                    nc.vector.tensor_mul(out=t1[:], in0=t1[:], in1=r_[:])
                    nc.vector.tensor_scalar(out=t1[:], in0=t1[:], scalar1=-0.5,
                                            scalar2=1.5, op0=ALU.mult, op1=ALU.add)
                    nc.vector.tensor_mul(out=r_[:], in0=r_[:], in1=t1[:])
                negms = sc.tile([P, 1], F32, tag="negms")
                nc.vector.tensor_scalar(out=negms[:], in0=mv[:, 0:1],
                                        scalar1=r_[:], scalar2=-1.0,
                                        op0=ALU.mult, op1=ALU.mult)

                # ---- LN apply on ACT (fp32 + bf16 copies)
                y32 = sc.tile([P, V], F32, tag="y32")
                nc.scalar.activation(out=y32[:], in_=ax[:], func=AF.Identity,
                                     bias=negms[:], scale=r_[:])
                yb = sc.tile([P, V], BF16, tag="yb")
                nc.scalar.activation(out=yb[:], in_=ax[:], func=AF.Identity,
                                     bias=negms[:], scale=r_[:])

                # ---- yb^T via PE
                ybt_ps = ps2.tile([P, V], BF16, tag="tps")
                for c in range(VC):
                    nc.tensor.transpose(ybt_ps[:, c * P:(c + 1) * P],
                                        yb[:, c * P:(c + 1) * P], ident[:])
                ybt = sc.tile([P, VC, P], BF16, tag="ybt")
                nc.vector.tensor_copy(ybt[:], ybt_ps[:])

                # ---- FFN
                h1_ps = ps.tile([P, P], F32, tag="qkt")
                for c in range(VC):
                    nc.tensor.matmul(h1_ps[:], w[f"w1{l}"][:, c, :],
                                     ybt[:, c, :],
                                     start=(c == 0), stop=(c == VC - 1))
                h1r = sc.tile([P, P], BF16, tag="h1r")
                nc.scalar.activation(out=h1r[:], in_=h1_ps[:], func=AF.Relu,
                                     bias=w[f"b1{l}"][:], scale=1.0)
                o2 = ps.tile([P, V], F32, tag="big")
                for off in range(0, V, 512):
                    n = min(512, V - off)
                    nc.tensor.matmul(o2[:, off:off + n], h1r[:],
                                     w[f"w2{l}"][:, off:off + n],
                                     start=True, stop=False)
                    nc.tensor.matmul(o2[:, off:off + n], onesrow[:],
                                     w[f"b2{l}"][:, off:off + n],
                                     start=False, stop=True)
                z32 = xb.tile([P, V], F32, tag="xo32")
                nc.vector.tensor_add(out=z32[:], in0=o2[:], in1=y32[:])

                if l == 2:
                    nc.sync.dma_start(zout[:, :], z32[:])
                    break

                zb = sc.tile([P, V], BF16, tag="zb")
                nc.scalar.activation(out=zb[:], in_=z32[:], func=AF.Copy)
                zbt_ps = ps2.tile([P, V], BF16, tag="tps")
                for c in range(VC):
                    nc.tensor.transpose(zbt_ps[:, c * P:(c + 1) * P],
                                        zb[:, c * P:(c + 1) * P], ident[:])
                zbt = xb.tile([P, VC, P], BF16, tag="xqt")
                nc.vector.tensor_copy(zbt[:], zbt_ps[:])

                # ---- AllGather pair exchange (natural + transposed)
                nc.sync.dma_start(agin[l][:, 0:V], zb[:])
                nc.sync.dma_start(agin[l][:, V:2 * V],
                                  zbt[:].rearrange("p c i -> p (c i)"))
                nc.gpsimd.collective_compute(
                    "AllGather", ALU.bypass, replica_groups=groups,
                    ins=[agin[l][:, :]], outs=[agout[l][:, :, :]])
                xf_n = xb.tile([P, 2, V], BF16, tag="xf")
                nc.sync.dma_start(
                    xf_n[:], agout[l][:, :, 0:V].rearrange("r p d -> p r d"))
                xft_n = xb.tile([P, VC, S], BF16, tag="xft")
                for r in range(2):
                    src = bass.AP(
                        tensor=agout[l], offset=V + r * P * 2 * V,
                        ap=[[2 * V, P], [P, VC], [1, P]])
                    nc.sync.dma_start(xft_n[:, :, r * P:(r + 1) * P], src)
                xo32, xqt, xf, xft = z32, zbt, xf_n, xft_n

    nc.compile()
    return nc


def _bf(a):
    return np.ascontiguousarray(a.astype(ml_dtypes.bfloat16))


def kernel(**inputs):
    X = np.asarray(inputs["X"], dtype=np.float32)
    lys = int(np.asarray(inputs["lys_pos"]))
    if "nc" not in _CACHE:
        _CACHE["nc"] = _build()
    nc = _CACHE["nc"]

    wshared = {}
    for l, li in enumerate((1, 2, 3)):
        Wq = np.asarray(inputs[f"Wq{li}"], np.float32)
        Wk = np.asarray(inputs[f"Wk{li}"], np.float32)
        wv = np.asarray(inputs[f"wv{li}"], np.float32)
        W1 = np.asarray(inputs[f"rW1_{li}"], np.float32)
        W2 = np.asarray(inputs[f"rW2_{li}"], np.float32)
        wshared[f"wq{l}"] = _bf(Wq.reshape(VC, P, H).transpose(1, 0, 2))
        wshared[f"wk{l}"] = _bf(Wk.reshape(VC, P, H).transpose(1, 0, 2))
        wvb = wv[:, None] * np.asarray(BCO, np.float32)[None, :]
        wshared[f"wvbp{l}"] = np.ascontiguousarray(wvb.astype(np.float32))
        wshared[f"wvbm{l}"] = np.ascontiguousarray((-wvb).astype(np.float32))
        wshared[f"w1{l}"] = _bf(W1.reshape(VC, P, H).transpose(1, 0, 2))
        wshared[f"b1{l}"] = np.ascontiguousarray(
            np.asarray(inputs[f"rb1_{li}"], np.float32)[:, None])
        wshared[f"w2{l}"] = _bf(W2)
        wshared[f"b2{l}"] = _bf(
            np.asarray(inputs[f"rb2_{li}"], np.float32)[None, :])

    in_maps = []
    for c in range(NCORES):
        b, h = c // 2, c % 2
        Xb = X[b]                        # [S, V]
        Xo = Xb[h * P:(h + 1) * P]       # [P, V]
        m = dict(wshared)
        m["xo32"] = np.ascontiguousarray(Xo)
        m["xqt"] = _bf(Xo.T.reshape(VC, P, P).transpose(1, 0, 2))
        m["xf"] = _bf(Xb.reshape(2, P, V).transpose(1, 0, 2))
        m["xft"] = _bf(Xb.T.reshape(VC, P, S).transpose(1, 0, 2))
        in_maps.append(m)

    res = run_bass_kernel_spmd(nc, in_maps, core_ids=list(range(NCORES)))
    _CACHE["last_res"] = res

    X3 = np.zeros((B, S, V), np.float32)
    for c in range(NCORES):
        b, h = c // 2, c % 2
        X3[b, h * P:(h + 1) * P] = res.results[c]["zout"]

    # ---- layer 4 + head on host (fp32) ----
    def ln(x):
        m_ = x.mean(-1, keepdims=True)
        v_ = ((x - m_) ** 2).mean(-1, keepdims=True)
        return (x - m_) / np.sqrt(v_ + EPS)

    Wq4 = np.asarray(inputs["Wq4"], np.float32)
    Wk4 = np.asarray(inputs["Wk4"], np.float32)
    wv4 = np.asarray(inputs["wv4"], np.float32)
    Xl = X3[:, lys, :][:, None, :]
    q = Xl @ Wq4
    k = X3 @ Wk4
    feat = np.tanh(q[:, :, None, :] + k[:, None, :, :])
    sco = np.einsum("bijh,h->bij", feat, wv4)
    sco = sco - sco.max(-1, keepdims=True)
    a = np.exp(sco)
    a /= a.sum(-1, keepdims=True)
    att = np.einsum("bij,bjd->bid", a, X3)
    Xl = ln(att + Xl)
    h_ = np.maximum(Xl @ np.asarray(inputs["hW1"], np.float32)
                    + np.asarray(inputs["hb1"], np.float32), 0.0)
    h_ = np.maximum(h_ @ np.asarray(inputs["hW2"], np.float32)
                    + np.asarray(inputs["hb2"], np.float32), 0.0)
    logits = (h_ @ np.asarray(inputs["hW3"], np.float32)
              + np.asarray(inputs["hb3"], np.float32))[:, 0, :]
    return logits.astype(np.float32)


# revision 8
# speedup vs baseline: 1.5235x; 1.5235x over previous
"""Trainium2 Bass kernel for the additive-attention transformer.

Sharding: 8 cores = (batch b in 0..3) x (sequence half in 0..1); each core
owns 128 query rows through 3 encoder layers; core pairs AllGather updated
halves after layers 1, 2.  The tiny layer-4 attention and the head run on
the host in fp32.

Scores trick: tanh(q+k) is approximated by an 8-term sine series
tanh(x) ~ sum_m b_m sin(m*w0*x) fitted on [-6.3, 6.3].  With the
phase-split identity
    sin(A+B) = sin(A+pi/4) sin(B+pi/4) - sin(A-pi/4) sin(B-pi/4)
the [S,S,H] additive-attention tensor never materializes: scores become 32
PE matmul chunks contracting over H between per-side sine features.  ACT's
sin table only covers ~[-3.4, 3.4], so harmonic angles are range-reduced
with batched add_range_wrap ops on DVE.  Softmax exp runs on ACT (costs
one activation-table switch each way per layer, cheaper than a DVE
polynomial chain).
"""

import numpy as np
import ml_dtypes

import concourse.bass as bass
import concourse.mybir as mybir
import concourse.tile as tile
from concourse import bacc
from concourse.bass_utils import run_bass_kernel_spmd
from concourse.masks import make_identity

F32 = mybir.dt.float32
BF16 = mybir.dt.bfloat16
AF = mybir.ActivationFunctionType
ALU = mybir.AluOpType

V, H, B, S = 1280, 128, 4, 256
P = 128
VC = V // P      # 10 v-chunks
NCORES = 8
EPS = 1e-5

M = 8                        # sine harmonics
NW = M - 1                   # wrapped harmonics (m = 2..M)
W0 = float(np.pi / 8.0)      # base frequency (L = 8)
PI = float(np.pi)

# tanh(x) ~ sum b[m-1] sin(m*W0*x), minimax fit on [-6.3, 6.3]
BCO = [1.2080011502433625, -0.017812034631637875, 0.26891009956897627,
       -0.009017248674699148, 0.0735473800612, 0.004038038433573973,
       0.014539648460423977, 0.007089646089277671]

_CACHE = {}


def _build():
    nc = bacc.Bacc("TRN2", target_bir_lowering=False, debug=False,
                   num_devices=NCORES)

    # ---- I/O ----
    xo32_in = nc.dram_tensor("xo32", [P, V], F32, kind="ExternalInput")
    xqt_in = nc.dram_tensor("xqt", [P, VC, P], BF16, kind="ExternalInput")
    xf_in = nc.dram_tensor("xf", [P, 2, V], BF16, kind="ExternalInput")
    xft_in = nc.dram_tensor("xft", [P, VC, S], BF16, kind="ExternalInput")
    w_in = {}
    for l in range(3):
        w_in[f"wq{l}"] = nc.dram_tensor(f"wq{l}", [P, VC, H], BF16, kind="ExternalInput")
        w_in[f"wk{l}"] = nc.dram_tensor(f"wk{l}", [P, VC, H], BF16, kind="ExternalInput")
        w_in[f"wvbp{l}"] = nc.dram_tensor(f"wvbp{l}", [P, M], F32, kind="ExternalInput")
        w_in[f"wvbm{l}"] = nc.dram_tensor(f"wvbm{l}", [P, M], F32, kind="ExternalInput")
        w_in[f"w1{l}"] = nc.dram_tensor(f"w1{l}", [P, VC, H], BF16, kind="ExternalInput")
        w_in[f"b1{l}"] = nc.dram_tensor(f"b1{l}", [P, 1], F32, kind="ExternalInput")
        w_in[f"w2{l}"] = nc.dram_tensor(f"w2{l}", [P, V], BF16, kind="ExternalInput")
        w_in[f"b2{l}"] = nc.dram_tensor(f"b2{l}", [1, V], BF16, kind="ExternalInput")
    zout = nc.dram_tensor("zout", [P, V], F32, kind="ExternalOutput")

    agin = [nc.dram_tensor(f"agin{l}", [P, V], BF16) for l in range(2)]
    agout = [nc.dram_tensor(f"agout{l}", [2, P, V], BF16) for l in range(2)]
    groups = [[0, 1], [2, 3], [4, 5], [6, 7]]

    with tile.TileContext(nc) as tc:
        with tc.tile_pool(name="persist", bufs=1) as pp, \
             tc.tile_pool(name="xbuf", bufs=2) as xb, \
             tc.tile_pool(name="scratch", bufs=2) as sc, \
             tc.tile_pool(name="feat", bufs=1) as fp_pool, \
             tc.tile_pool(name="ps", bufs=1, space="PSUM") as ps, \
             tc.tile_pool(name="ps2", bufs=1, space="PSUM") as ps2:

            ident = pp.tile([P, P], BF16, tag="ident")
            make_identity(nc, ident[:])
            ones = pp.tile([P, 1], BF16, tag="ones")
            nc.vector.memset(ones[:], 1.0)
            onesrow = pp.tile([1, P], BF16, tag="onesrow")
            nc.vector.memset(onesrow[:], 1.0)
            biasp = pp.tile([P, 1], F32, tag="biasp")
            nc.vector.memset(biasp[:], PI / 4.0)
            biasm = pp.tile([P, 1], F32, tag="biasm")
            nc.vector.memset(biasm[:], -PI / 4.0)
            # m*w0 harmonic-scale tile for the one-op angle build
            mwt = pp.tile([P, NW, 3 * P], F32, tag="mwt")
            for mi in range(NW):
                nc.vector.memset(mwt[:, mi, :], float((mi + 2) * W0))

            # initial X buffers first (critical path), then weights by layer
            xo32 = xb.tile([P, V], F32, tag="xo32")
            nc.sync.dma_start(xo32[:], xo32_in[:, :])
            xqt = xb.tile([P, VC, P], BF16, tag="xqt")
            nc.sync.dma_start(xqt[:], xqt_in[:, :, :])
            xf = xb.tile([P, 2, V], BF16, tag="xf")
            nc.sync.dma_start(xf[:], xf_in[:, :, :])
            xft = xb.tile([P, VC, S], BF16, tag="xft")
            nc.sync.dma_start(xft[:], xft_in[:, :, :])

            w = {}
            for l in range(3):
                for pre in ("wq", "wk", "wvbp", "wvbm", "w1", "b1", "w2", "b2"):
                    k = f"{pre}{l}"
                    t = w_in[k]
                    tl = pp.tile(list(t.shape), t.dtype, tag=k)
                    nc.sync.dma_start(tl[:], t[(slice(None),) * len(t.shape)])
                    w[k] = tl

            # broadcast +-b_m*wv columns into [P, M, P] scale tiles (one-time)
            wvbf = {}
            for l in range(3):
                for sgn in ("p", "m"):
                    tl = pp.tile([P, M, P], BF16, tag=f"wvbf{sgn}{l}")
                    nc.vector.tensor_copy(
                        tl[:],
                        w[f"wvb{sgn}{l}"][:].unsqueeze(2).broadcast_to((P, M, P)))
                    wvbf[f"{sgn}{l}"] = tl

            for l in range(3):
                # ---- projections: qkt = [qT own | kT abs] in one PSUM tile
                qkt = ps.tile([P, 3 * P], F32, tag="qkt")
                for c in range(VC):
                    nc.tensor.matmul(qkt[:, 0:P], w[f"wq{l}"][:, c, :],
                                     xqt[:, c, :],
                                     start=(c == 0), stop=(c == VC - 1))
                for c in range(VC):
                    nc.tensor.matmul(qkt[:, P:3 * P], w[f"wk{l}"][:, c, :],
                                     xft[:, c, :],
                                     start=(c == 0), stop=(c == VC - 1))
                qkt32 = sc.tile([P, 3 * P], F32, tag="qkt32")
                nc.vector.tensor_copy(qkt32[:], qkt[:])

                # ---- harmonic angles m=2..M in one broadcast multiply
                th = fp_pool.tile([P, NW, 3 * P], F32, tag="th", name=f"th_{l}")
                thw = fp_pool.tile([P, NW, 3 * P], F32, tag="thw", name=f"thw_{l}")
                nc.vector.tensor_mul(
                    out=th[:],
                    in0=qkt32[:].unsqueeze(1).broadcast_to((P, NW, 3 * P)),
                    in1=mwt[:])
                # wrap into [-pi, pi]: m<=5 one wrap, m>=6 two wraps
                nc.vector.add_range_wrap(out=thw[:, 0:4, :], in_=th[:, 0:4, :],
                                         shift=0.0, bound=PI, period=2 * PI)
                nc.vector.add_range_wrap(out=thw[:, 4:NW, :], in_=th[:, 4:NW, :],
                                         shift=0.0, bound=3 * PI, period=4 * PI)
                nc.vector.add_range_wrap(out=thw[:, 4:NW, :], in_=thw[:, 4:NW, :],
                                         shift=0.0, bound=PI, period=2 * PI)

                # ---- features sin(theta +- pi/4), bf16
                fpl = fp_pool.tile([P, M, 3 * P], BF16, tag="fpl", name=f"fpl_{l}")
                fml = fp_pool.tile([P, M, 3 * P], BF16, tag="fml", name=f"fml_{l}")
                nc.scalar.activation(out=fpl[:, 0, :], in_=qkt32[:], func=AF.Sin,
                                     scale=W0, bias=biasp[:])
                nc.scalar.activation(out=fml[:, 0, :], in_=qkt32[:], func=AF.Sin,
                                     scale=W0, bias=biasm[:])
                nc.scalar.activation(out=fpl[:, 1:M, :], in_=thw[:], func=AF.Sin,
                                     bias=biasp[:])
                nc.scalar.activation(out=fml[:, 1:M, :], in_=thw[:], func=AF.Sin,
                                     bias=biasm[:])

                # ---- q-side scaling by +-b_m * wv (two batched multiplies)
                qfp = fp_pool.tile([P, M, P], BF16, tag="qfp", name=f"qfp_{l}")
                qfm = fp_pool.tile([P, M, P], BF16, tag="qfm", name=f"qfm_{l}")
                nc.vector.tensor_mul(out=qfp[:], in0=fpl[:, :, 0:P],
                                     in1=wvbf[f"p{l}"][:])
                nc.vector.tensor_mul(out=qfm[:], in0=fml[:, :, 0:P],
                                     in1=wvbf[f"m{l}"][:])

                # ---- scores^T [j(half h), i] accumulated over 2M chunks
                sct = ps.tile([P, S], F32, tag="sct")
                for h in range(2):
                    for mi in range(M):
                        nc.tensor.matmul(sct[:, h * P:(h + 1) * P],
                                         fpl[:, mi, (1 + h) * P:(2 + h) * P],
                                         qfp[:, mi, :],
                                         start=(mi == 0), stop=False)
                    for mi in range(M):
                        nc.tensor.matmul(sct[:, h * P:(h + 1) * P],
                                         fml[:, mi, (1 + h) * P:(2 + h) * P],
                                         qfm[:, mi, :],
                                         start=False, stop=(mi == M - 1))

                # ---- softmax exp on ACT (one table switch each way)
                expt = sc.tile([P, S], BF16, tag="expt")
                nc.scalar.activation(out=expt[:], in_=sct[:], func=AF.Exp)

                # ---- sums + attnV
                sums = ps.tile([P, 1], F32, tag="sct")
                for h in range(2):
                    nc.tensor.matmul(sums[:], expt[:, h * P:(h + 1) * P],
                                     ones[:], start=(h == 0), stop=(h == 1))
                rin = sc.tile([P, 1], F32, tag="rin")
                nc.vector.reciprocal(rin[:], sums[:])

                av = ps.tile([P, V], F32, tag="big")
                for off in range(0, V, 512):
                    n = min(512, V - off)
                    for h in range(2):
                        nc.tensor.matmul(av[:, off:off + n],
                                         expt[:, h * P:(h + 1) * P],
                                         xf[:, h, off:off + n],
                                         start=(h == 0), stop=(h == 1))

                # ---- ax = av/sums + X, LN
                ax = sc.tile([P, V], F32, tag="ax")
                nc.vector.scalar_tensor_tensor(out=ax[:], in0=av[:],
                                               scalar=rin[:], in1=xo32[:],
                                               op0=ALU.mult, op1=ALU.add)
                stats = sc.tile([P, 5, 6], F32, tag="stats")
                axg = ax[:].rearrange("p (n s) -> p n s", s=256)
                for g in range(5):
                    nc.vector.bn_stats(out=stats[:, g, :], in_=axg[:, g, :])
                mv = sc.tile([P, 2], F32, tag="mv")
                nc.vector.bn_aggr(out=mv[:], in_=stats[:])
                vv = sc.tile([P, 1], F32, tag="vv")
                nc.vector.tensor_scalar(out=vv[:], in0=mv[:, 1:2], scalar1=EPS,
                                        scalar2=None, op0=ALU.add)
                s_ = sc.tile([P, 1], F32, tag="s_")
                nc.vector.reciprocal(s_[:], vv[:])
                r_ = sc.tile([P, 1], F32, tag="r_")
                nc.vector.tensor_scalar(out=r_[:], in0=s_[:], scalar1=0.4315,
                                        scalar2=0.361, op0=ALU.mult, op1=ALU.add)
                t1 = sc.tile([P, 1], F32, tag="t1")
                for _ in range(3):
                    nc.vector.tensor_mul(out=t1[:], in0=vv[:], in1=r_[:])
                    nc.vector.tensor_mul(out=t1[:], in0=t1[:], in1=r_[:])
                    nc.vector.tensor_scalar(out=t1[:], in0=t1[:], scalar1=-0.5,
                                            scalar2=1.5, op0=ALU.mult, op1=ALU.add)
                    nc.vector.tensor_mul(out=r_[:], in0=r_[:], in1=t1[:])
                negms = sc.tile([P, 1], F32, tag="negms")
                nc.vector.tensor_scalar(out=negms[:], in0=mv[:, 0:1],
                                        scalar1=r_[:], scalar2=-1.0,
                                        op0=ALU.mult, op1=ALU.mult)

                # ---- LN apply on ACT (fp32 + bf16 copies)
                y32 = sc.tile([P, V], F32, tag="y32")
                nc.scalar.activation(out=y32[:], in_=ax[:], func=AF.Identity,
                                     bias=negms[:], scale=r_[:])
                yb = sc.tile([P, V], BF16, tag="yb")
                nc.scalar.activation(out=yb[:], in_=ax[:], func=AF.Identity,
                                     bias=negms[:], scale=r_[:])

                # ---- yb^T via PE (copy back on ACT)
                ybt_ps = ps2.tile([P, V], BF16, tag="tps")
                for c in range(VC):
                    nc.tensor.transpose(ybt_ps[:, c * P:(c + 1) * P],
                                        yb[:, c * P:(c + 1) * P], ident[:])
                ybt = sc.tile([P, VC, P], BF16, tag="ybt")
                nc.scalar.activation(out=ybt[:], in_=ybt_ps[:], func=AF.Copy)

                # ---- FFN
                h1_ps = ps.tile([P, P], F32, tag="qkt")
                for c in range(VC):
                    nc.tensor.matmul(h1_ps[:], w[f"w1{l}"][:, c, :],
                                     ybt[:, c, :],
                                     start=(c == 0), stop=(c == VC - 1))
                h1r = sc.tile([P, P], BF16, tag="h1r")
                nc.scalar.activation(out=h1r[:], in_=h1_ps[:], func=AF.Relu,
                                     bias=w[f"b1{l}"][:], scale=1.0)
                o2 = ps.tile([P, V], F32, tag="big")
                for off in range(0, V, 512):
                    n = min(512, V - off)
                    nc.tensor.matmul(o2[:, off:off + n], h1r[:],
                                     w[f"w2{l}"][:, off:off + n],
                                     start=True, stop=False)
                    nc.tensor.matmul(o2[:, off:off + n], onesrow[:],
                                     w[f"b2{l}"][:, off:off + n],
                                     start=False, stop=True)
                z32 = xb.tile([P, V], F32, tag="xo32")
                nc.vector.tensor_add(out=z32[:], in0=o2[:], in1=y32[:])

                if l == 2:
                    nc.sync.dma_start(zout[:, :], z32[:])
                    break

                zb = sc.tile([P, V], BF16, tag="zb")
                nc.scalar.activation(out=zb[:], in_=z32[:], func=AF.Copy)
                zbt_ps = ps2.tile([P, V], BF16, tag="tps")
                for c in range(VC):
                    nc.tensor.transpose(zbt_ps[:, c * P:(c + 1) * P],
                                        zb[:, c * P:(c + 1) * P], ident[:])
                zbt = xb.tile([P, VC, P], BF16, tag="xqt")
                nc.scalar.activation(out=zbt[:], in_=zbt_ps[:], func=AF.Copy)

                # ---- AllGather natural halves only; transpose locally
                nc.sync.dma_start(agin[l][:, :], zb[:])
                nc.gpsimd.collective_compute(
                    "AllGather", ALU.bypass, replica_groups=groups,
                    ins=[agin[l][:, :]], outs=[agout[l][:, :, :]])
                xf_n = xb.tile([P, 2, V], BF16, tag="xf")
                nc.sync.dma_start(
                    xf_n[:], agout[l][:, :, :].rearrange("r p d -> p r d"))
                xft_n = xb.tile([P, VC, S], BF16, tag="xft")
                for r in range(2):
                    tr_ps = ps2.tile([P, V], BF16, tag="tps")
                    for c in range(VC):
                        nc.tensor.transpose(tr_ps[:, c * P:(c + 1) * P],
                                            xf_n[:, r, c * P:(c + 1) * P],
                                            ident[:])
                    nc.vector.tensor_copy(
                        xft_n[:, :, r * P:(r + 1) * P],
                        tr_ps[:].rearrange("p (c i) -> p c i", i=P))
                xo32, xqt, xf, xft = z32, zbt, xf_n, xft_n

    nc.compile()
    return nc


def _bf(a):
    return np.ascontiguousarray(a.astype(ml_dtypes.bfloat16))


def kernel(**inputs):
    X = np.asarray(inputs["X"], dtype=np.float32)
    lys = int(np.asarray(inputs["lys_pos"]))
    if "nc" not in _CACHE:
        _CACHE["nc"] = _build()
    nc = _CACHE["nc"]

    wshared = {}
    for l, li in enumerate((1, 2, 3)):
        Wq = np.asarray(inputs[f"Wq{li}"], np.float32)
        Wk = np.asarray(inputs[f"Wk{li}"], np.float32)
        wv = np.asarray(inputs[f"wv{li}"], np.float32)
        W1 = np.asarray(inputs[f"rW1_{li}"], np.float32)
        W2 = np.asarray(inputs[f"rW2_{li}"], np.float32)
        wshared[f"wq{l}"] = _bf(Wq.reshape(VC, P, H).transpose(1, 0, 2))
        wshared[f"wk{l}"] = _bf(Wk.reshape(VC, P, H).transpose(1, 0, 2))
        wvb = wv[:, None] * np.asarray(BCO, np.float32)[None, :]
        wshared[f"wvbp{l}"] = np.ascontiguousarray(wvb.astype(np.float32))
        wshared[f"wvbm{l}"] = np.ascontiguousarray((-wvb).astype(np.float32))
        wshared[f"w1{l}"] = _bf(W1.reshape(VC, P, H).transpose(1, 0, 2))
        wshared[f"b1{l}"] = np.ascontiguousarray(
            np.asarray(inputs[f"rb1_{li}"], np.float32)[:, None])
        wshared[f"w2{l}"] = _bf(W2)
        wshared[f"b2{l}"] = _bf(
            np.asarray(inputs[f"rb2_{li}"], np.float32)[None, :])

    in_maps = []
    for c in range(NCORES):
        b, h = c // 2, c % 2
        Xb = X[b]                        # [S, V]
        Xo = Xb[h * P:(h + 1) * P]       # [P, V]
        m = dict(wshared)
        m["xo32"] = np.ascontiguousarray(Xo)
        m["xqt"] = _bf(Xo.T.reshape(VC, P, P).transpose(1, 0, 2))
        m["xf"] = _bf(Xb.reshape(2, P, V).transpose(1, 0, 2))
        m["xft"] = _bf(Xb.T.reshape(VC, P, S).transpose(1, 0, 2))
        in_maps.append(m)

    res = run_bass_kernel_spmd(nc, in_maps, core_ids=list(range(NCORES)))
    _CACHE["last_res"] = res

    X3 = np.zeros((B, S, V), np.float32)
    for c in range(NCORES):
        b, h = c // 2, c % 2
        X3[b, h * P:(h + 1) * P] = res.results[c]["zout"]

    # ---- layer 4 + head on host (fp32) ----
    def ln(x):
        m_ = x.mean(-1, keepdims=True)
        v_ = ((x - m_) ** 2).mean(-1, keepdims=True)
        return (x - m_) / np.sqrt(v_ + EPS)

    Wq4 = np.asarray(inputs["Wq4"], np.float32)
    Wk4 = np.asarray(inputs["Wk4"], np.float32)
    wv4 = np.asarray(inputs["wv4"], np.float32)
    Xl = X3[:, lys, :][:, None, :]
    q = Xl @ Wq4
    k = X3 @ Wk4
    feat = np.tanh(q[:, :, None, :] + k[:, None, :, :])
    sco = np.einsum("bijh,h->bij", feat, wv4)
    sco = sco - sco.max(-1, keepdims=True)
    a = np.exp(sco)
    a /= a.sum(-1, keepdims=True)
    att = np.einsum("bij,bjd->bid", a, X3)
    Xl = ln(att + Xl)
    h_ = np.maximum(Xl @ np.asarray(inputs["hW1"], np.float32)
                    + np.asarray(inputs["hb1"], np.float32), 0.0)
    h_ = np.maximum(h_ @ np.asarray(inputs["hW2"], np.float32)
                    + np.asarray(inputs["hb2"], np.float32), 0.0)
    logits = (h_ @ np.asarray(inputs["hW3"], np.float32)
              + np.asarray(inputs["hb3"], np.float32))[:, 0, :]
    return logits.astype(np.float32)


# revision 10
# speedup vs baseline: 1.5840x; 1.0397x over previous
"""Trainium2 Bass kernel for the additive-attention transformer.

Sharding: 8 cores = (batch b in 0..3) x (sequence half in 0..1); each core
owns 128 query rows through 3 encoder layers; core pairs AllGather updated
halves after layers 1, 2.  The tiny layer-4 attention and the head run on
the host in fp32.

Scores trick: tanh(q+k) is approximated by an 8-term sine series
tanh(x) ~ sum_m b_m sin(m*w0*x) fitted on [-6.3, 6.3].  With the
phase-split identity
    sin(A+B) = sin(A+pi/4) sin(B+pi/4) - sin(A-pi/4) sin(B-pi/4)
the [S,S,H] additive-attention tensor never materializes: scores become 16
PE matmul chunks contracting over H between per-side sine features.  ACT's
sin table only covers ~[-3.4, 3.4], so harmonic angles are range-reduced
with batched add_range_wrap ops on DVE.  Softmax exp runs on ACT with a
fused row-sum (accum_out); the q-side feature pipeline of layer l+1 is
emitted before layer l's AllGather so it executes in the collective's
shadow (engine queues are in-order).
"""

import numpy as np
import ml_dtypes

import concourse.bass as bass
import concourse.mybir as mybir
import concourse.tile as tile
from concourse import bacc
from concourse.bass_utils import run_bass_kernel_spmd
from concourse.masks import make_identity

F32 = mybir.dt.float32
BF16 = mybir.dt.bfloat16
AF = mybir.ActivationFunctionType
ALU = mybir.AluOpType

V, H, B, S = 1280, 128, 4, 256
P = 128
VC = V // P      # 10 v-chunks
NCORES = 8
EPS = 1e-5

M = 8                        # sine harmonics
NW = M - 1                   # wrapped harmonics (m = 2..M)
W0 = float(np.pi / 8.0)      # base frequency (L = 8)
PI = float(np.pi)

# tanh(x) ~ sum b[m-1] sin(m*W0*x), minimax fit on [-6.3, 6.3]
BCO = [1.2080011502433625, -0.017812034631637875, 0.26891009956897627,
       -0.009017248674699148, 0.0735473800612, 0.004038038433573973,
       0.014539648460423977, 0.007089646089277671]

_CACHE = {}


def _build():
    nc = bacc.Bacc("TRN2", target_bir_lowering=False, debug=False,
                   num_devices=NCORES)

    # ---- I/O ----
    xo32_in = nc.dram_tensor("xo32", [P, V], F32, kind="ExternalInput")
    xqt_in = nc.dram_tensor("xqt", [P, VC, P], BF16, kind="ExternalInput")
    xf_in = nc.dram_tensor("xf", [P, 2, V], BF16, kind="ExternalInput")
    xft_in = nc.dram_tensor("xft", [P, VC, S], BF16, kind="ExternalInput")
    w_in = {}
    for l in range(3):
        w_in[f"wq{l}"] = nc.dram_tensor(f"wq{l}", [P, VC, H], BF16, kind="ExternalInput")
        w_in[f"wk{l}"] = nc.dram_tensor(f"wk{l}", [P, VC, H], BF16, kind="ExternalInput")
        w_in[f"wvbp{l}"] = nc.dram_tensor(f"wvbp{l}", [P, M], F32, kind="ExternalInput")
        w_in[f"wvbm{l}"] = nc.dram_tensor(f"wvbm{l}", [P, M], F32, kind="ExternalInput")
        w_in[f"w1{l}"] = nc.dram_tensor(f"w1{l}", [P, VC, H], BF16, kind="ExternalInput")
        w_in[f"b1{l}"] = nc.dram_tensor(f"b1{l}", [P, 1], F32, kind="ExternalInput")
        w_in[f"w2{l}"] = nc.dram_tensor(f"w2{l}", [P, V], BF16, kind="ExternalInput")
        w_in[f"b2{l}"] = nc.dram_tensor(f"b2{l}", [1, V], BF16, kind="ExternalInput")
    zout = nc.dram_tensor("zout", [P, V], F32, kind="ExternalOutput")

    agin = [nc.dram_tensor(f"agin{l}", [P, V], BF16) for l in range(2)]
    agout = [nc.dram_tensor(f"agout{l}", [2, P, V], BF16) for l in range(2)]
    groups = [[0, 1], [2, 3], [4, 5], [6, 7]]

    with tile.TileContext(nc) as tc:
        with tc.tile_pool(name="persist", bufs=1) as pp, \
             tc.tile_pool(name="xbuf", bufs=2) as xb, \
             tc.tile_pool(name="scratch", bufs=2) as sc, \
             tc.tile_pool(name="feat", bufs=2) as fq_pool, \
             tc.tile_pool(name="featk", bufs=1) as fk_pool, \
             tc.tile_pool(name="ps", bufs=1, space="PSUM") as ps, \
             tc.tile_pool(name="ps2", bufs=1, space="PSUM") as ps2:

            ident = pp.tile([P, P], BF16, tag="ident")
            make_identity(nc, ident[:])
            onesrow = pp.tile([1, P], BF16, tag="onesrow")
            nc.vector.memset(onesrow[:], 1.0)
            biasp = pp.tile([P, 1], F32, tag="biasp")
            nc.vector.memset(biasp[:], PI / 4.0)
            biasm = pp.tile([P, 1], F32, tag="biasm")
            nc.vector.memset(biasm[:], -PI / 4.0)
            # m*w0 harmonic-scale tile for the one-op angle builds
            mwt = pp.tile([P, NW, 2 * P], F32, tag="mwt")
            for mi in range(NW):
                nc.vector.memset(mwt[:, mi, :], float((mi + 2) * W0))

            # initial X buffers first (critical path), then weights by layer
            xo32 = xb.tile([P, V], F32, tag="xo32")
            nc.sync.dma_start(xo32[:], xo32_in[:, :])
            xqt = xb.tile([P, VC, P], BF16, tag="xqt")
            nc.sync.dma_start(xqt[:], xqt_in[:, :, :])
            xf = xb.tile([P, 2, V], BF16, tag="xf")
            nc.sync.dma_start(xf[:], xf_in[:, :, :])
            xft = xb.tile([P, VC, S], BF16, tag="xft")
            nc.sync.dma_start(xft[:], xft_in[:, :, :])

            w = {}
            for l in range(3):
                for pre in ("wq", "wk", "wvbp", "wvbm", "w1", "b1", "w2", "b2"):
                    k = f"{pre}{l}"
                    t = w_in[k]
                    tl = pp.tile(list(t.shape), t.dtype, tag=k)
                    nc.sync.dma_start(tl[:], t[(slice(None),) * len(t.shape)])
                    w[k] = tl

            # broadcast +-b_m*wv columns into [P, M, P] scale tiles (one-time)
            wvbf = {}
            for l in range(3):
                for sgn in ("p", "m"):
                    tl = pp.tile([P, M, P], BF16, tag=f"wvbf{sgn}{l}")
                    nc.vector.tensor_copy(
                        tl[:],
                        w[f"wvb{sgn}{l}"][:].unsqueeze(2).broadcast_to((P, M, P)))
                    wvbf[f"{sgn}{l}"] = tl

            def emit_q_side(l, xqt_t):
                """Project q and build the scaled q-features for layer l.
                Emitted before layer l-1's AllGather: no AG dependency."""
                qt_ps = ps.tile([P, P], F32, tag="qt", name=f"qt_{l}")
                for c in range(VC):
                    nc.tensor.matmul(qt_ps[:], w[f"wq{l}"][:, c, :],
                                     xqt_t[:, c, :],
                                     start=(c == 0), stop=(c == VC - 1))
                qt32 = sc.tile([P, P], F32, tag="qt32", name=f"qt32_{l}")
                nc.vector.tensor_copy(qt32[:], qt_ps[:])
                thq = fq_pool.tile([P, NW, P], F32, tag="thq", name=f"thq_{l}")
                nc.vector.tensor_mul(
                    out=thq[:],
                    in0=qt32[:].unsqueeze(1).broadcast_to((P, NW, P)),
                    in1=mwt[:, :, 0:P])
                nc.vector.add_range_wrap(out=thq[:, 0:4, :], in_=thq[:, 0:4, :],
                                         shift=0.0, bound=PI, period=2 * PI)
                nc.vector.add_range_wrap(out=thq[:, 4:NW, :], in_=thq[:, 4:NW, :],
                                         shift=0.0, bound=3 * PI, period=4 * PI)
                nc.vector.add_range_wrap(out=thq[:, 4:NW, :], in_=thq[:, 4:NW, :],
                                         shift=0.0, bound=PI, period=2 * PI)
                fqp = fq_pool.tile([P, M, P], BF16, tag="fqp", name=f"fqp_{l}")
                fqm = fq_pool.tile([P, M, P], BF16, tag="fqm", name=f"fqm_{l}")
                nc.scalar.activation(out=fqp[:, 0, :], in_=qt32[:], func=AF.Sin,
                                     scale=W0, bias=biasp[:])
                nc.scalar.activation(out=fqm[:, 0, :], in_=qt32[:], func=AF.Sin,
                                     scale=W0, bias=biasm[:])
                nc.scalar.activation(out=fqp[:, 1:M, :], in_=thq[:], func=AF.Sin,
                                     bias=biasp[:])
                nc.scalar.activation(out=fqm[:, 1:M, :], in_=thq[:], func=AF.Sin,
                                     bias=biasm[:])
                qfp = fq_pool.tile([P, M, P], BF16, tag="qfp", name=f"qfp_{l}")
                qfm = fq_pool.tile([P, M, P], BF16, tag="qfm", name=f"qfm_{l}")
                nc.vector.tensor_mul(out=qfp[:], in0=fqp[:], in1=wvbf[f"p{l}"][:])
                nc.vector.tensor_mul(out=qfm[:], in0=fqm[:], in1=wvbf[f"m{l}"][:])
                return qfp, qfm

            qfp, qfm = emit_q_side(0, xqt)

            for l in range(3):
                # ---- k projection (needs AllGathered X^T)
                kt_ps = ps.tile([P, S], F32, tag="kt")
                for c in range(VC):
                    nc.tensor.matmul(kt_ps[:], w[f"wk{l}"][:, c, :],
                                     xft[:, c, :],
                                     start=(c == 0), stop=(c == VC - 1))
                kt32 = sc.tile([P, S], F32, tag="kt32")
                nc.vector.tensor_copy(kt32[:], kt_ps[:])

                thk = fk_pool.tile([P, NW, S], F32, tag="thk", name=f"thk_{l}")
                nc.vector.tensor_mul(
                    out=thk[:],
                    in0=kt32[:].unsqueeze(1).broadcast_to((P, NW, S)),
                    in1=mwt[:])
                nc.vector.add_range_wrap(out=thk[:, 0:4, :], in_=thk[:, 0:4, :],
                                         shift=0.0, bound=PI, period=2 * PI)
                nc.vector.add_range_wrap(out=thk[:, 4:NW, :], in_=thk[:, 4:NW, :],
                                         shift=0.0, bound=3 * PI, period=4 * PI)
                nc.vector.add_range_wrap(out=thk[:, 4:NW, :], in_=thk[:, 4:NW, :],
                                         shift=0.0, bound=PI, period=2 * PI)

                fkp = fk_pool.tile([P, M, S], BF16, tag="fkp", name=f"fkp_{l}")
                fkm = fk_pool.tile([P, M, S], BF16, tag="fkm", name=f"fkm_{l}")
                nc.scalar.activation(out=fkp[:, 0, :], in_=kt32[:], func=AF.Sin,
                                     scale=W0, bias=biasp[:])
                nc.scalar.activation(out=fkm[:, 0, :], in_=kt32[:], func=AF.Sin,
                                     scale=W0, bias=biasm[:])
                nc.scalar.activation(out=fkp[:, 1:M, :], in_=thk[:], func=AF.Sin,
                                     bias=biasp[:])
                nc.scalar.activation(out=fkm[:, 1:M, :], in_=thk[:], func=AF.Sin,
                                     bias=biasm[:])

                # ---- scores [i, j] accumulated over 2M chunks
                sct = ps.tile([P, S], F32, tag="sct")
                for mi in range(M):
                    nc.tensor.matmul(sct[:], qfp[:, mi, :], fkp[:, mi, :],
                                     start=(mi == 0), stop=False)
                for mi in range(M):
                    nc.tensor.matmul(sct[:], qfm[:, mi, :], fkm[:, mi, :],
                                     start=False, stop=(mi == M - 1))

                # ---- softmax exp on ACT with fused row-sum
                expt = sc.tile([P, S], BF16, tag="expt")
                sums = sc.tile([P, 1], F32, tag="sums")
                nc.scalar.activation(out=expt[:], in_=sct[:], func=AF.Exp,
                                     accum_out=sums[:])
                rin = sc.tile([P, 1], F32, tag="rin")
                nc.vector.reciprocal(rin[:], sums[:])

                # ---- exp^T for attnV (PE transpose, ACT copy back)
                expT_ps = ps.tile([P, 2, P], BF16, tag="kt")
                for h in range(2):
                    nc.tensor.transpose(expT_ps[:, h, :],
                                        expt[:, h * P:(h + 1) * P], ident[:])
                expT = sc.tile([P, 2, P], BF16, tag="expT")
                nc.scalar.activation(out=expT[:], in_=expT_ps[:], func=AF.Copy)

                av = ps.tile([P, V], F32, tag="big")
                for off in range(0, V, 512):
                    n = min(512, V - off)
                    for h in range(2):
                        nc.tensor.matmul(av[:, off:off + n], expT[:, h, :],
                                         xf[:, h, off:off + n],
                                         start=(h == 0), stop=(h == 1))

                # ---- ax = av/sums + X, LN
                ax = sc.tile([P, V], F32, tag="ax")
                nc.vector.scalar_tensor_tensor(out=ax[:], in0=av[:],
                                               scalar=rin[:], in1=xo32[:],
                                               op0=ALU.mult, op1=ALU.add)
                stats = sc.tile([P, 3, 6], F32, tag="stats")
                nc.vector.bn_stats(out=stats[:, 0, :], in_=ax[:, 0:512])
                nc.vector.bn_stats(out=stats[:, 1, :], in_=ax[:, 512:1024])
                nc.vector.bn_stats(out=stats[:, 2, :], in_=ax[:, 1024:1280])
                mv = sc.tile([P, 2], F32, tag="mv")
                nc.vector.bn_aggr(out=mv[:], in_=stats[:])
                vv = sc.tile([P, 1], F32, tag="vv")
                nc.vector.tensor_scalar(out=vv[:], in0=mv[:, 1:2], scalar1=EPS,
                                        scalar2=None, op0=ALU.add)
                s_ = sc.tile([P, 1], F32, tag="s_")
                nc.vector.reciprocal(s_[:], vv[:])
                r_ = sc.tile([P, 1], F32, tag="r_")
                nc.vector.tensor_scalar(out=r_[:], in0=s_[:], scalar1=0.4315,
                                        scalar2=0.361, op0=ALU.mult, op1=ALU.add)
                t1 = sc.tile([P, 1], F32, tag="t1")
                for _ in range(3):
                    nc.vector.tensor_mul(out=t1[:], in0=vv[:], in1=r_[:])
                    nc.vector.tensor_mul(out=t1[:], in0=t1[:], in1=r_[:])
                    nc.vector.tensor_scalar(out=t1[:], in0=t1[:], scalar1=-0.5,
                                            scalar2=1.5, op0=ALU.mult, op1=ALU.add)
                    nc.vector.tensor_mul(out=r_[:], in0=r_[:], in1=t1[:])
                negms = sc.tile([P, 1], F32, tag="negms")
                nc.vector.tensor_scalar(out=negms[:], in0=mv[:, 0:1],
                                        scalar1=r_[:], scalar2=-1.0,
                                        op0=ALU.mult, op1=ALU.mult)

                # ---- LN apply on ACT (fp32 + bf16 copies)
                y32 = sc.tile([P, V], F32, tag="y32")
                nc.scalar.activation(out=y32[:], in_=ax[:], func=AF.Identity,
                                     bias=negms[:], scale=r_[:])
                yb = sc.tile([P, V], BF16, tag="yb")
                nc.scalar.activation(out=yb[:], in_=ax[:], func=AF.Identity,
                                     bias=negms[:], scale=r_[:])

                # ---- yb^T via PE (copy back on DVE)
                ybt_ps = ps2.tile([P, V], BF16, tag="tps")
                for c in range(VC):
                    nc.tensor.transpose(ybt_ps[:, c * P:(c + 1) * P],
                                        yb[:, c * P:(c + 1) * P], ident[:])
                ybt = sc.tile([P, VC, P], BF16, tag="ybt")
                nc.vector.tensor_copy(ybt[:], ybt_ps[:])

                # ---- FFN
                h1_ps = ps.tile([P, P], F32, tag="qt")
                for c in range(VC):
                    nc.tensor.matmul(h1_ps[:], w[f"w1{l}"][:, c, :],
                                     ybt[:, c, :],
                                     start=(c == 0), stop=(c == VC - 1))
                h1r = sc.tile([P, P], BF16, tag="h1r")
                nc.scalar.activation(out=h1r[:], in_=h1_ps[:], func=AF.Relu,
                                     bias=w[f"b1{l}"][:], scale=1.0)
                o2 = ps.tile([P, V], F32, tag="big")
                for off in range(0, V, 512):
                    n = min(512, V - off)
                    nc.tensor.matmul(o2[:, off:off + n], h1r[:],
                                     w[f"w2{l}"][:, off:off + n],
                                     start=True, stop=False)
                    nc.tensor.matmul(o2[:, off:off + n], onesrow[:],
                                     w[f"b2{l}"][:, off:off + n],
                                     start=False, stop=True)
                z32 = xb.tile([P, V], F32, tag="xo32")
                nc.vector.tensor_add(out=z32[:], in0=o2[:], in1=y32[:])

                if l == 2:
                    nc.sync.dma_start(zout[:, :], z32[:])
                    break

                zb = sc.tile([P, V], BF16, tag="zb")
                nc.scalar.activation(out=zb[:], in_=z32[:], func=AF.Copy)
                zbt_ps = ps2.tile([P, V], BF16, tag="tps")
                for c in range(VC):
                    nc.tensor.transpose(zbt_ps[:, c * P:(c + 1) * P],
                                        zb[:, c * P:(c + 1) * P], ident[:])
                zbt = xb.tile([P, VC, P], BF16, tag="xqt")
                nc.vector.tensor_copy(zbt[:], zbt_ps[:])

                # next layer's q side runs in the AllGather's shadow
                qfp, qfm = emit_q_side(l + 1, zbt)

                # ---- AllGather natural halves only; transpose locally
                nc.sync.dma_start(agin[l][:, :], zb[:])
                nc.gpsimd.collective_compute(
                    "AllGather", ALU.bypass, replica_groups=groups,
                    ins=[agin[l][:, :]], outs=[agout[l][:, :, :]])
                xf_n = xb.tile([P, 2, V], BF16, tag="xf")
                nc.sync.dma_start(
                    xf_n[:], agout[l][:, :, :].rearrange("r p d -> p r d"))
                xft_n = xb.tile([P, VC, S], BF16, tag="xft")
                for r in range(2):
                    tr_ps = ps2.tile([P, V], BF16, tag="tps")
                    for c in range(VC):
                        nc.tensor.transpose(tr_ps[:, c * P:(c + 1) * P],
                                            xf_n[:, r, c * P:(c + 1) * P],
                                            ident[:])
                    nc.vector.tensor_copy(
                        xft_n[:, :, r * P:(r + 1) * P],
                        tr_ps[:].rearrange("p (c i) -> p c i", i=P))
                xo32, xf, xft = z32, xf_n, xft_n

    nc.compile()
    return nc


def _bf(a):
    return np.ascontiguousarray(a.astype(ml_dtypes.bfloat16))


def kernel(**inputs):
    X = np.asarray(inputs["X"], dtype=np.float32)
    lys = int(np.asarray(inputs["lys_pos"]))
    if "nc" not in _CACHE:
        _CACHE["nc"] = _build()
    nc = _CACHE["nc"]

    wshared = {}
    for l, li in enumerate((1, 2, 3)):
        Wq = np.asarray(inputs[f"Wq{li}"], np.float32)
        Wk = np.asarray(inputs[f"Wk{li}"], np.float32)
        wv = np.asarray(inputs[f"wv{li}"], np.float32)
        W1 = np.asarray(inputs[f"rW1_{li}"], np.float32)
        W2 = np.asarray(inputs[f"rW2_{li}"], np.float32)
        wshared[f"wq{l}"] = _bf(Wq.reshape(VC, P, H).transpose(1, 0, 2))
        wshared[f"wk{l}"] = _bf(Wk.reshape(VC, P, H).transpose(1, 0, 2))
        wvb = wv[:, None] * np.asarray(BCO, np.float32)[None, :]
        wshared[f"wvbp{l}"] = np.ascontiguousarray(wvb.astype(np.float32))
        wshared[f"wvbm{l}"] = np.ascontiguousarray((-wvb).astype(np.float32))
        wshared[f"w1{l}"] = _bf(W1.reshape(VC, P, H).transpose(1, 0, 2))
        wshared[f"b1{l}"] = np.ascontiguousarray(
            np.asarray(inputs[f"rb1_{li}"], np.float32)[:, None])
        wshared[f"w2{l}"] = _bf(W2)
        wshared[f"b2{l}"] = _bf(
            np.asarray(inputs[f"rb2_{li}"], np.float32)[None, :])

    in_maps = []
    for c in range(NCORES):
        b, h = c // 2, c % 2
        Xb = X[b]                        # [S, V]
        Xo = Xb[h * P:(h + 1) * P]       # [P, V]
        m = dict(wshared)
        m["xo32"] = np.ascontiguousarray(Xo)
        m["xqt"] = _bf(Xo.T.reshape(VC, P, P).transpose(1, 0, 2))
        m["xf"] = _bf(Xb.reshape(2, P, V).transpose(1, 0, 2))
        m["xft"] = _bf(Xb.T.reshape(VC, P, S).transpose(1, 0, 2))
        in_maps.append(m)

    res = run_bass_kernel_spmd(nc, in_maps, core_ids=list(range(NCORES)))
    _CACHE["last_res"] = res

    X3 = np.zeros((B, S, V), np.float32)
    for c in range(NCORES):
        b, h = c // 2, c % 2
        X3[b, h * P:(h + 1) * P] = res.results[c]["zout"]

    # ---- layer 4 + head on host (fp32) ----
    def ln(x):
        m_ = x.mean(-1, keepdims=True)
        v_ = ((x - m_) ** 2).mean(-1, keepdims=True)
        return (x - m_) / np.sqrt(v_ + EPS)

    Wq4 = np.asarray(inputs["Wq4"], np.float32)
    Wk4 = np.asarray(inputs["Wk4"], np.float32)
    wv4 = np.asarray(inputs["wv4"], np.float32)
    Xl = X3[:, lys, :][:, None, :]
    q = Xl @ Wq4
    k = X3 @ Wk4
    feat = np.tanh(q[:, :, None, :] + k[:, None, :, :])
    sco = np.einsum("bijh,h->bij", feat, wv4)
    sco = sco - sco.max(-1, keepdims=True)
    a = np.exp(sco)
    a /= a.sum(-1, keepdims=True)
    att = np.einsum("bij,bjd->bid", a, X3)
    Xl = ln(att + Xl)
    h_ = np.maximum(Xl @ np.asarray(inputs["hW1"], np.float32)
                    + np.asarray(inputs["hb1"], np.float32), 0.0)
    h_ = np.maximum(h_ @ np.asarray(inputs["hW2"], np.float32)
                    + np.asarray(inputs["hb2"], np.float32), 0.0)
    logits = (h_ @ np.asarray(inputs["hW3"], np.float32)
              + np.asarray(inputs["hb3"], np.float32))[:, 0, :]
    return logits.astype(np.float32)


# revision 11
# speedup vs baseline: 1.6265x; 1.0268x over previous
"""Trainium2 Bass kernel for the additive-attention transformer.

Sharding: 8 cores = (batch b in 0..3) x (sequence half in 0..1); each core
owns 128 query rows through 3 encoder layers; core pairs AllGather updated
halves after layers 1, 2.  The tiny layer-4 attention and the head run on
the host in fp32.

Scores trick: tanh(q+k) is approximated by an 8-term sine series
tanh(x) ~ sum_m b_m sin(m*w0*x) fitted on [-6.3, 6.3].  With the
phase-split identity
    sin(A+B) = sin(A+pi/4) sin(B+pi/4) - sin(A-pi/4) sin(B-pi/4)
the [S,S,H] additive-attention tensor never materializes: scores become 16
PE matmul chunks contracting over H between per-side sine features.  ACT's
sin table only covers ~[-3.4, 3.4], so harmonic angles are range-reduced
with batched add_range_wrap ops on DVE.  Softmax exp runs on ACT with a
fused row-sum (accum_out); the q-side feature pipeline of layer l+1 is
emitted before layer l's AllGather so it executes in the collective's
shadow (engine queues are in-order).
"""

import numpy as np
import ml_dtypes

import concourse.bass as bass
import concourse.mybir as mybir
import concourse.tile as tile
from concourse import bacc
from concourse.bass_utils import run_bass_kernel_spmd
from concourse.masks import make_identity

F32 = mybir.dt.float32
BF16 = mybir.dt.bfloat16
AF = mybir.ActivationFunctionType
ALU = mybir.AluOpType

V, H, B, S = 1280, 128, 4, 256
P = 128
VC = V // P      # 10 v-chunks
NCORES = 8
EPS = 1e-5

M = 8                        # sine harmonics
NW = M - 2                   # wrapped harmonics (m = 3..M)
W0 = float(np.pi / 8.0)      # base frequency (L = 8)
PI = float(np.pi)

# tanh(x) ~ sum b[m-1] sin(m*W0*x), minimax fit on [-6.3, 6.3]
BCO = [1.2080011502433625, -0.017812034631637875, 0.26891009956897627,
       -0.009017248674699148, 0.0735473800612, 0.004038038433573973,
       0.014539648460423977, 0.007089646089277671]

_CACHE = {}


def _build():
    nc = bacc.Bacc("TRN2", target_bir_lowering=False, debug=False,
                   num_devices=NCORES)

    # ---- I/O ----
    xo32_in = nc.dram_tensor("xo32", [P, V], F32, kind="ExternalInput")
    xqt_in = nc.dram_tensor("xqt", [P, VC, P], BF16, kind="ExternalInput")
    xf_in = nc.dram_tensor("xf", [P, 2, V], BF16, kind="ExternalInput")
    xft_in = nc.dram_tensor("xft", [P, VC, S], BF16, kind="ExternalInput")
    w_in = {}
    for l in range(3):
        w_in[f"wq{l}"] = nc.dram_tensor(f"wq{l}", [P, VC, H], BF16, kind="ExternalInput")
        w_in[f"wk{l}"] = nc.dram_tensor(f"wk{l}", [P, VC, H], BF16, kind="ExternalInput")
        w_in[f"wvbp{l}"] = nc.dram_tensor(f"wvbp{l}", [P, M], F32, kind="ExternalInput")
        w_in[f"wvbm{l}"] = nc.dram_tensor(f"wvbm{l}", [P, M], F32, kind="ExternalInput")
        w_in[f"w1{l}"] = nc.dram_tensor(f"w1{l}", [P, VC, H], BF16, kind="ExternalInput")
        w_in[f"b1{l}"] = nc.dram_tensor(f"b1{l}", [P, 1], F32, kind="ExternalInput")
        w_in[f"w2{l}"] = nc.dram_tensor(f"w2{l}", [P, V], BF16, kind="ExternalInput")
        w_in[f"b2{l}"] = nc.dram_tensor(f"b2{l}", [1, V], BF16, kind="ExternalInput")
    zout = nc.dram_tensor("zout", [P, V], F32, kind="ExternalOutput")

    agin = [nc.dram_tensor(f"agin{l}", [P, V], BF16) for l in range(2)]
    agout = [nc.dram_tensor(f"agout{l}", [2, P, V], BF16) for l in range(2)]
    groups = [[0, 1], [2, 3], [4, 5], [6, 7]]

    with tile.TileContext(nc) as tc:
        with tc.tile_pool(name="persist", bufs=1) as pp, \
             tc.tile_pool(name="xbuf", bufs=2) as xb, \
             tc.tile_pool(name="scratch", bufs=2) as sc, \
             tc.tile_pool(name="feat", bufs=2) as fq_pool, \
             tc.tile_pool(name="featk", bufs=1) as fk_pool, \
             tc.tile_pool(name="ps", bufs=1, space="PSUM") as ps, \
             tc.tile_pool(name="ps2", bufs=1, space="PSUM") as ps2:

            ident = pp.tile([P, P], BF16, tag="ident")
            make_identity(nc, ident[:])
            onesrow = pp.tile([1, P], BF16, tag="onesrow")
            nc.vector.memset(onesrow[:], 1.0)
            biasp = pp.tile([P, 1], F32, tag="biasp")
            nc.vector.memset(biasp[:], PI / 4.0)
            biasm = pp.tile([P, 1], F32, tag="biasm")
            nc.vector.memset(biasm[:], -PI / 4.0)
            # m*w0 harmonic-scale tile for the one-op angle builds
            mwt = pp.tile([P, NW, 2 * P], F32, tag="mwt")
            for mi in range(NW):
                nc.vector.memset(mwt[:, mi, :], float((mi + 3) * W0))

            # initial X buffers first (critical path), then weights by layer
            xqt = xb.tile([P, VC, P], BF16, tag="xqt")
            nc.sync.dma_start(xqt[:], xqt_in[:, :, :])
            xft = xb.tile([P, VC, S], BF16, tag="xft")
            nc.sync.dma_start(xft[:], xft_in[:, :, :])
            xf = xb.tile([P, 2, V], BF16, tag="xf")
            nc.sync.dma_start(xf[:], xf_in[:, :, :])
            xo32 = xb.tile([P, V], F32, tag="xo32")
            nc.sync.dma_start(xo32[:], xo32_in[:, :])

            w = {}
            for l in range(3):
                for pre in ("wq", "wk", "wvbp", "wvbm", "w1", "b1", "w2", "b2"):
                    k = f"{pre}{l}"
                    t = w_in[k]
                    tl = pp.tile(list(t.shape), t.dtype, tag=k)
                    nc.sync.dma_start(tl[:], t[(slice(None),) * len(t.shape)])
                    w[k] = tl

            # broadcast +-b_m*wv columns into [P, M, P] scale tiles (one-time)
            wvbf = {}
            for l in range(3):
                for sgn in ("p", "m"):
                    tl = pp.tile([P, M, P], BF16, tag=f"wvbf{sgn}{l}")
                    nc.vector.tensor_copy(
                        tl[:],
                        w[f"wvb{sgn}{l}"][:].unsqueeze(2).broadcast_to((P, M, P)))
                    wvbf[f"{sgn}{l}"] = tl

            def emit_q_side(l, xqt_t):
                """Project q and build the scaled q-features for layer l.
                Emitted before layer l-1's AllGather: no AG dependency."""
                qt_ps = ps.tile([P, P], F32, tag="qt", name=f"qt_{l}")
                for c in range(VC):
                    nc.tensor.matmul(qt_ps[:], w[f"wq{l}"][:, c, :],
                                     xqt_t[:, c, :],
                                     start=(c == 0), stop=(c == VC - 1))
                qt32 = sc.tile([P, P], F32, tag="qt32", name=f"qt32_{l}")
                nc.scalar.activation(out=qt32[:], in_=qt_ps[:], func=AF.Copy)
                thq = fq_pool.tile([P, NW, P], F32, tag="thq", name=f"thq_{l}")
                nc.vector.tensor_mul(
                    out=thq[:],
                    in0=qt32[:].unsqueeze(1).broadcast_to((P, NW, P)),
                    in1=mwt[:, :, 0:P])
                nc.vector.add_range_wrap(out=thq[:, 0:3, :], in_=thq[:, 0:3, :],
                                         shift=0.0, bound=PI, period=2 * PI)
                nc.vector.add_range_wrap(out=thq[:, 3:NW, :], in_=thq[:, 3:NW, :],
                                         shift=0.0, bound=3 * PI, period=4 * PI)
                nc.vector.add_range_wrap(out=thq[:, 3:NW, :], in_=thq[:, 3:NW, :],
                                         shift=0.0, bound=PI, period=2 * PI)
                fqp = fq_pool.tile([P, M, P], BF16, tag="fqp", name=f"fqp_{l}")
                fqm = fq_pool.tile([P, M, P], BF16, tag="fqm", name=f"fqm_{l}")
                nc.scalar.activation(out=fqp[:, 0, :], in_=qt32[:], func=AF.Sin,
                                     scale=W0, bias=biasp[:])
                nc.scalar.activation(out=fqm[:, 0, :], in_=qt32[:], func=AF.Sin,
                                     scale=W0, bias=biasm[:])
                nc.scalar.activation(out=fqp[:, 1, :], in_=qt32[:], func=AF.Sin,
                                     scale=2 * W0, bias=biasp[:])
                nc.scalar.activation(out=fqm[:, 1, :], in_=qt32[:], func=AF.Sin,
                                     scale=2 * W0, bias=biasm[:])
                nc.scalar.activation(out=fqp[:, 2:M, :], in_=thq[:], func=AF.Sin,
                                     bias=biasp[:])
                nc.scalar.activation(out=fqm[:, 2:M, :], in_=thq[:], func=AF.Sin,
                                     bias=biasm[:])
                qfp = fq_pool.tile([P, M, P], BF16, tag="qfp", name=f"qfp_{l}")
                qfm = fq_pool.tile([P, M, P], BF16, tag="qfm", name=f"qfm_{l}")
                nc.vector.tensor_mul(out=qfp[:], in0=fqp[:], in1=wvbf[f"p{l}"][:])
                nc.vector.tensor_mul(out=qfm[:], in0=fqm[:], in1=wvbf[f"m{l}"][:])
                return qfp, qfm

            qfp, qfm = emit_q_side(0, xqt)

            for l in range(3):
                # ---- k projection (needs AllGathered X^T)
                kt_ps = ps.tile([P, S], F32, tag="kt")
                for c in range(VC):
                    nc.tensor.matmul(kt_ps[:], w[f"wk{l}"][:, c, :],
                                     xft[:, c, :],
                                     start=(c == 0), stop=(c == VC - 1))
                kt32 = sc.tile([P, S], F32, tag="kt32")
                nc.scalar.activation(out=kt32[:], in_=kt_ps[:], func=AF.Copy)

                thk = fk_pool.tile([P, NW, S], F32, tag="thk", name=f"thk_{l}")
                nc.vector.tensor_mul(
                    out=thk[:],
                    in0=kt32[:].unsqueeze(1).broadcast_to((P, NW, S)),
                    in1=mwt[:])
                nc.vector.add_range_wrap(out=thk[:, 0:3, :], in_=thk[:, 0:3, :],
                                         shift=0.0, bound=PI, period=2 * PI)
                nc.vector.add_range_wrap(out=thk[:, 3:NW, :], in_=thk[:, 3:NW, :],
                                         shift=0.0, bound=3 * PI, period=4 * PI)
                nc.vector.add_range_wrap(out=thk[:, 3:NW, :], in_=thk[:, 3:NW, :],
                                         shift=0.0, bound=PI, period=2 * PI)

                fkp = fk_pool.tile([P, M, S], BF16, tag="fkp", name=f"fkp_{l}")
                fkm = fk_pool.tile([P, M, S], BF16, tag="fkm", name=f"fkm_{l}")
                nc.scalar.activation(out=fkp[:, 0, :], in_=kt32[:], func=AF.Sin,
                                     scale=W0, bias=biasp[:])
                nc.scalar.activation(out=fkm[:, 0, :], in_=kt32[:], func=AF.Sin,
                                     scale=W0, bias=biasm[:])
                nc.scalar.activation(out=fkp[:, 1, :], in_=kt32[:], func=AF.Sin,
                                     scale=2 * W0, bias=biasp[:])
                nc.scalar.activation(out=fkm[:, 1, :], in_=kt32[:], func=AF.Sin,
                                     scale=2 * W0, bias=biasm[:])
                nc.scalar.activation(out=fkp[:, 2:M, :], in_=thk[:], func=AF.Sin,
                                     bias=biasp[:])
                nc.scalar.activation(out=fkm[:, 2:M, :], in_=thk[:], func=AF.Sin,
                                     bias=biasm[:])

                # ---- scores [i, j] accumulated over 2M chunks
                sct = ps.tile([P, S], F32, tag="sct")
                for mi in range(M):
                    nc.tensor.matmul(sct[:], qfp[:, mi, :], fkp[:, mi, :],
                                     start=(mi == 0), stop=False)
                for mi in range(M):
                    nc.tensor.matmul(sct[:], qfm[:, mi, :], fkm[:, mi, :],
                                     start=False, stop=(mi == M - 1))

                # ---- softmax exp on ACT with fused row-sum
                expt = sc.tile([P, S], BF16, tag="expt")
                sums = sc.tile([P, 1], F32, tag="sums")
                nc.scalar.activation(out=expt[:], in_=sct[:], func=AF.Exp,
                                     accum_out=sums[:])
                rin = sc.tile([P, 1], F32, tag="rin")
                nc.vector.reciprocal(rin[:], sums[:])

                # ---- exp^T for attnV (PE transpose, ACT copy back)
                expT_ps = ps.tile([P, 2, P], BF16, tag="kt")
                for h in range(2):
                    nc.tensor.transpose(expT_ps[:, h, :],
                                        expt[:, h * P:(h + 1) * P], ident[:])
                expT = sc.tile([P, 2, P], BF16, tag="expT")
                nc.scalar.activation(out=expT[:], in_=expT_ps[:], func=AF.Copy)

                av = ps.tile([P, V], F32, tag="big")
                for off in range(0, V, 512):
                    n = min(512, V - off)
                    for h in range(2):
                        nc.tensor.matmul(av[:, off:off + n], expT[:, h, :],
                                         xf[:, h, off:off + n],
                                         start=(h == 0), stop=(h == 1))

                # ---- ax = av/sums + X, LN
                ax = sc.tile([P, V], F32, tag="ax")
                nc.vector.scalar_tensor_tensor(out=ax[:], in0=av[:],
                                               scalar=rin[:], in1=xo32[:],
                                               op0=ALU.mult, op1=ALU.add)
                stats = sc.tile([P, 3, 6], F32, tag="stats")
                nc.vector.bn_stats(out=stats[:, 0, :], in_=ax[:, 0:512])
                nc.vector.bn_stats(out=stats[:, 1, :], in_=ax[:, 512:1024])
                nc.vector.bn_stats(out=stats[:, 2, :], in_=ax[:, 1024:1280])
                mv = sc.tile([P, 2], F32, tag="mv")
                nc.vector.bn_aggr(out=mv[:], in_=stats[:])
                vv = sc.tile([P, 1], F32, tag="vv")
                nc.vector.tensor_scalar(out=vv[:], in0=mv[:, 1:2], scalar1=EPS,
                                        scalar2=None, op0=ALU.add)
                s_ = sc.tile([P, 1], F32, tag="s_")
                nc.vector.reciprocal(s_[:], vv[:])
                r_ = sc.tile([P, 1], F32, tag="r_")
                nc.vector.tensor_scalar(out=r_[:], in0=s_[:], scalar1=0.4315,
                                        scalar2=0.361, op0=ALU.mult, op1=ALU.add)
                t1 = sc.tile([P, 1], F32, tag="t1")
                for _ in range(3):
                    nc.vector.tensor_mul(out=t1[:], in0=vv[:], in1=r_[:])
                    nc.vector.tensor_mul(out=t1[:], in0=t1[:], in1=r_[:])
                    nc.vector.tensor_scalar(out=t1[:], in0=t1[:], scalar1=-0.5,
                                            scalar2=1.5, op0=ALU.mult, op1=ALU.add)
                    nc.vector.tensor_mul(out=r_[:], in0=r_[:], in1=t1[:])
                negms = sc.tile([P, 1], F32, tag="negms")
                nc.vector.tensor_scalar(out=negms[:], in0=mv[:, 0:1],
                                        scalar1=r_[:], scalar2=-1.0,
                                        op0=ALU.mult, op1=ALU.mult)

                # ---- LN apply on ACT (fp32 + bf16 copies)
                yb = sc.tile([P, V], BF16, tag="yb")
                nc.scalar.activation(out=yb[:], in_=ax[:], func=AF.Identity,
                                     bias=negms[:], scale=r_[:])
                y32 = sc.tile([P, V], F32, tag="y32")
                nc.scalar.activation(out=y32[:], in_=ax[:], func=AF.Identity,
                                     bias=negms[:], scale=r_[:])

                # ---- yb^T via PE (copy back on DVE)
                ybt_ps = ps2.tile([P, V], BF16, tag="tps")
                for c in range(VC):
                    nc.tensor.transpose(ybt_ps[:, c * P:(c + 1) * P],
                                        yb[:, c * P:(c + 1) * P], ident[:])
                ybt = sc.tile([P, VC, P], BF16, tag="ybt")
                nc.vector.tensor_copy(ybt[:], ybt_ps[:])

                # ---- FFN
                h1_ps = ps.tile([P, P], F32, tag="qt")
                for c in range(VC):
                    nc.tensor.matmul(h1_ps[:], w[f"w1{l}"][:, c, :],
                                     ybt[:, c, :],
                                     start=(c == 0), stop=(c == VC - 1))
                h1r = sc.tile([P, P], BF16, tag="h1r")
                nc.scalar.activation(out=h1r[:], in_=h1_ps[:], func=AF.Relu,
                                     bias=w[f"b1{l}"][:], scale=1.0)
                o2 = ps.tile([P, V], F32, tag="big")
                for off in range(0, V, 512):
                    n = min(512, V - off)
                    nc.tensor.matmul(o2[:, off:off + n], h1r[:],
                                     w[f"w2{l}"][:, off:off + n],
                                     start=True, stop=False)
                    nc.tensor.matmul(o2[:, off:off + n], onesrow[:],
                                     w[f"b2{l}"][:, off:off + n],
                                     start=False, stop=True)
                z32 = xb.tile([P, V], F32, tag="xo32")
                nc.vector.tensor_add(out=z32[:], in0=o2[:], in1=y32[:])

                if l == 2:
                    nc.sync.dma_start(zout[:, :], z32[:])
                    break

                zb = sc.tile([P, V], BF16, tag="zb")
                nc.scalar.activation(out=zb[:], in_=z32[:], func=AF.Copy)
                zbt_ps = ps2.tile([P, V], BF16, tag="tps")
                for c in range(VC):
                    nc.tensor.transpose(zbt_ps[:, c * P:(c + 1) * P],
                                        zb[:, c * P:(c + 1) * P], ident[:])
                zbt = xb.tile([P, VC, P], BF16, tag="xqt")
                nc.vector.tensor_copy(zbt[:], zbt_ps[:])

                # next layer's q side runs in the AllGather's shadow
                qfp, qfm = emit_q_side(l + 1, zbt)

                # ---- AllGather natural halves only; transpose locally
                nc.sync.dma_start(agin[l][:, :], zb[:])
                nc.gpsimd.collective_compute(
                    "AllGather", ALU.bypass, replica_groups=groups,
                    ins=[agin[l][:, :]], outs=[agout[l][:, :, :]])
                xf_n = xb.tile([P, 2, V], BF16, tag="xf")
                nc.sync.dma_start(
                    xf_n[:], agout[l][:, :, :].rearrange("r p d -> p r d"))
                xft_n = xb.tile([P, VC, S], BF16, tag="xft")
                for r in range(2):
                    tr_ps = ps2.tile([P, V], BF16, tag="tps")
                    for c in range(VC):
                        nc.tensor.transpose(tr_ps[:, c * P:(c + 1) * P],
                                            xf_n[:, r, c * P:(c + 1) * P],
                                            ident[:])
                    nc.scalar.activation(
                        out=xft_n[:, :, r * P:(r + 1) * P],
                        in_=tr_ps[:].rearrange("p (c i) -> p c i", i=P),
                        func=AF.Copy)
                xo32, xf, xft = z32, xf_n, xft_n

    nc.compile()
    return nc


def _bf(a):
    return np.ascontiguousarray(a.astype(ml_dtypes.bfloat16))


def kernel(**inputs):
    X = np.asarray(inputs["X"], dtype=np.float32)
    lys = int(np.asarray(inputs["lys_pos"]))
    if "nc" not in _CACHE:
        _CACHE["nc"] = _build()
    nc = _CACHE["nc"]

    wshared = {}
    for l, li in enumerate((1, 2, 3)):
        Wq = np.asarray(inputs[f"Wq{li}"], np.float32)
        Wk = np.asarray(inputs[f"Wk{li}"], np.float32)
        wv = np.asarray(inputs[f"wv{li}"], np.float32)
        W1 = np.asarray(inputs[f"rW1_{li}"], np.float32)
        W2 = np.asarray(inputs[f"rW2_{li}"], np.float32)
        wshared[f"wq{l}"] = _bf(Wq.reshape(VC, P, H).transpose(1, 0, 2))
        wshared[f"wk{l}"] = _bf(Wk.reshape(VC, P, H).transpose(1, 0, 2))
        wvb = wv[:, None] * np.asarray(BCO, np.float32)[None, :]
        wshared[f"wvbp{l}"] = np.ascontiguousarray(wvb.astype(np.float32))
        wshared[f"wvbm{l}"] = np.ascontiguousarray((-wvb).astype(np.float32))
        wshared[f"w1{l}"] = _bf(W1.reshape(VC, P, H).transpose(1, 0, 2))
        wshared[f"b1{l}"] = np.ascontiguousarray(
            np.asarray(inputs[f"rb1_{li}"], np.float32)[:, None])
        wshared[f"w2{l}"] = _bf(W2)
        wshared[f"b2{l}"] = _bf(
            np.asarray(inputs[f"rb2_{li}"], np.float32)[None, :])

    in_maps = []
    for c in range(NCORES):
        b, h = c // 2, c % 2
        Xb = X[b]                        # [S, V]
        Xo = Xb[h * P:(h + 1) * P]       # [P, V]
        m = dict(wshared)
        m["xo32"] = np.ascontiguousarray(Xo)
        m["xqt"] = _bf(Xo.T.reshape(VC, P, P).transpose(1, 0, 2))
        m["xf"] = _bf(Xb.reshape(2, P, V).transpose(1, 0, 2))
        m["xft"] = _bf(Xb.T.reshape(VC, P, S).transpose(1, 0, 2))
        in_maps.append(m)

    res = run_bass_kernel_spmd(nc, in_maps, core_ids=list(range(NCORES)))
    _CACHE["last_res"] = res

    X3 = np.zeros((B, S, V), np.float32)
    for c in range(NCORES):
        b, h = c // 2, c % 2
        X3[b, h * P:(h + 1) * P] = res.results[c]["zout"]

    # ---- layer 4 + head on host (fp32) ----
    def ln(x):
        m_ = x.mean(-1, keepdims=True)
        v_ = ((x - m_) ** 2).mean(-1, keepdims=True)
        return (x - m_) / np.sqrt(v_ + EPS)

    Wq4 = np.asarray(inputs["Wq4"], np.float32)
    Wk4 = np.asarray(inputs["Wk4"], np.float32)
    wv4 = np.asarray(inputs["wv4"], np.float32)
    Xl = X3[:, lys, :][:, None, :]
    q = Xl @ Wq4
    k = X3 @ Wk4
    feat = np.tanh(q[:, :, None, :] + k[:, None, :, :])
    sco = np.einsum("bijh,h->bij", feat, wv4)
    sco = sco - sco.max(-1, keepdims=True)
    a = np.exp(sco)
    a /= a.sum(-1, keepdims=True)
    att = np.einsum("bij,bjd->bid", a, X3)
    Xl = ln(att + Xl)
    h_ = np.maximum(Xl @ np.asarray(inputs["hW1"], np.float32)
                    + np.asarray(inputs["hb1"], np.float32), 0.0)
    h_ = np.maximum(h_ @ np.asarray(inputs["hW2"], np.float32)
                    + np.asarray(inputs["hb2"], np.float32), 0.0)
    logits = (h_ @ np.asarray(inputs["hW3"], np.float32)
              + np.asarray(inputs["hb3"], np.float32))[:, 0, :]
    return logits.astype(np.float32)


# revision 12
# speedup vs baseline: 1.8644x; 1.1463x over previous
"""Trainium2 Bass kernel for the additive-attention transformer.

Sharding: 8 cores = (batch b in 0..3) x (sequence half in 0..1); each core
owns 128 query rows through 3 encoder layers; core pairs AllGather updated
halves after layers 1, 2.  The tiny layer-4 attention and the head run on
the host in fp32.

Scores trick: tanh(q+k) is approximated by an 8-term sine series
tanh(x) ~ sum_m b_m sin(m*w0*x) fitted on [-6.3, 6.3].  With the
phase-split identity
    sin(A+B) = sin(A+pi/4) sin(B+pi/4) - sin(A-pi/4) sin(B-pi/4)
the [S,S,H] additive-attention tensor never materializes: scores become 16
PE matmul chunks contracting over H between per-side sine features.  ACT's
sin table only covers ~[-3.4, 3.4], so harmonic angles are range-reduced
with batched add_range_wrap ops on DVE.  Softmax exp runs on ACT with a
fused row-sum (accum_out); the q-side feature pipeline of layer l+1 is
emitted before layer l's AllGather so it executes in the collective's
shadow (engine queues are in-order).
"""

import numpy as np
import ml_dtypes

import concourse.bass as bass
import concourse.mybir as mybir
import concourse.tile as tile
from concourse import bacc
from concourse.bass_utils import run_bass_kernel_spmd
from concourse.masks import make_identity

F32 = mybir.dt.float32
BF16 = mybir.dt.bfloat16
AF = mybir.ActivationFunctionType
ALU = mybir.AluOpType

V, H, B, S = 1280, 128, 4, 256
P = 128
VC = V // P      # 10 v-chunks
NCORES = 8
EPS = 1e-5

M = 8                        # sine harmonics
NW = M - 2                   # wrapped harmonics (m = 3..M)
W0 = float(np.pi / 8.0)      # base frequency (L = 8)
PI = float(np.pi)

# tanh(x) ~ sum b[m-1] sin(m*W0*x), minimax fit on [-6.3, 6.3]
BCO = [1.2080011502433625, -0.017812034631637875, 0.26891009956897627,
       -0.009017248674699148, 0.0735473800612, 0.004038038433573973,
       0.014539648460423977, 0.007089646089277671]

_CACHE = {}


def _build():
    nc = bacc.Bacc("TRN2", target_bir_lowering=False, debug=False,
                   num_devices=NCORES)

    # ---- I/O ----
    xo32_in = nc.dram_tensor("xo32", [P, V], F32, kind="ExternalInput")
    xqt_in = nc.dram_tensor("xqt", [P, VC, P], BF16, kind="ExternalInput")
    xf_in = nc.dram_tensor("xf", [P, 2, V], BF16, kind="ExternalInput")
    xft_in = nc.dram_tensor("xft", [P, VC, S], BF16, kind="ExternalInput")
    w_in = {}
    for l in range(3):
        w_in[f"wq{l}"] = nc.dram_tensor(f"wq{l}", [P, VC, H], BF16, kind="ExternalInput")
        w_in[f"wk{l}"] = nc.dram_tensor(f"wk{l}", [P, VC, H], BF16, kind="ExternalInput")
        w_in[f"wvbp{l}"] = nc.dram_tensor(f"wvbp{l}", [P, M], F32, kind="ExternalInput")
        w_in[f"wvbm{l}"] = nc.dram_tensor(f"wvbm{l}", [P, M], F32, kind="ExternalInput")
        w_in[f"w1{l}"] = nc.dram_tensor(f"w1{l}", [P, VC, H], BF16, kind="ExternalInput")
        w_in[f"b1{l}"] = nc.dram_tensor(f"b1{l}", [P, 1], F32, kind="ExternalInput")
        w_in[f"w2{l}"] = nc.dram_tensor(f"w2{l}", [P, V], BF16, kind="ExternalInput")
        w_in[f"b2{l}"] = nc.dram_tensor(f"b2{l}", [1, V], BF16, kind="ExternalInput")
    zout = nc.dram_tensor("zout", [P, V], F32, kind="ExternalOutput")

    agin = [nc.dram_tensor(f"agin{l}", [P, V], BF16) for l in range(2)]
    agout = [nc.dram_tensor(f"agout{l}", [2, P, V], BF16) for l in range(2)]
    wuin = nc.dram_tensor("wuin", [1, 64], BF16)
    wuout = nc.dram_tensor("wuout", [2, 1, 64], BF16)
    groups = [[0, 1], [2, 3], [4, 5], [6, 7]]

    with tile.TileContext(nc) as tc:
        with tc.tile_pool(name="persist", bufs=1) as pp, \
             tc.tile_pool(name="xbuf", bufs=2) as xb, \
             tc.tile_pool(name="scratch", bufs=2) as sc, \
             tc.tile_pool(name="feat", bufs=2) as fq_pool, \
             tc.tile_pool(name="featk", bufs=1) as fk_pool, \
             tc.tile_pool(name="ps", bufs=1, space="PSUM") as ps, \
             tc.tile_pool(name="ps2", bufs=1, space="PSUM") as ps2:

            ident = pp.tile([P, P], BF16, tag="ident")
            make_identity(nc, ident[:])
            onesrow = pp.tile([1, P], BF16, tag="onesrow")
            nc.vector.memset(onesrow[:], 1.0)
            biasp = pp.tile([P, 1], F32, tag="biasp")
            nc.vector.memset(biasp[:], PI / 4.0)
            biasm = pp.tile([P, 1], F32, tag="biasm")
            nc.vector.memset(biasm[:], -PI / 4.0)
            # m*w0 harmonic-scale tile for the one-op angle builds
            mwt = pp.tile([P, NW, 2 * P], F32, tag="mwt")
            for mi in range(NW):
                nc.vector.memset(mwt[:, mi, :], float((mi + 3) * W0))

            # initial X buffers first (critical path), then weights by layer
            xqt = xb.tile([P, VC, P], BF16, tag="xqt")
            nc.sync.dma_start(xqt[:], xqt_in[:, :, :])
            xft = xb.tile([P, VC, S], BF16, tag="xft")
            nc.sync.dma_start(xft[:], xft_in[:, :, :])
            xf = xb.tile([P, 2, V], BF16, tag="xf")
            nc.sync.dma_start(xf[:], xf_in[:, :, :])
            xo32 = xb.tile([P, V], F32, tag="xo32")
            nc.sync.dma_start(xo32[:], xo32_in[:, :])

            # prime the CC ring while inputs stream in
            wut = pp.tile([1, 64], BF16, tag="wut")
            nc.vector.memset(wut[:], 0.0)
            nc.sync.dma_start(wuin[:, :], wut[:])
            nc.gpsimd.collective_compute(
                "AllGather", ALU.bypass, replica_groups=groups,
                ins=[wuin[:, :]], outs=[wuout[:, :, :]])

            w = {}
            for l in range(3):
                for pre in ("wq", "wk", "wvbp", "wvbm", "w1", "b1", "w2", "b2"):
                    k = f"{pre}{l}"
                    t = w_in[k]
                    tl = pp.tile(list(t.shape), t.dtype, tag=k)
                    nc.sync.dma_start(tl[:], t[(slice(None),) * len(t.shape)])
                    w[k] = tl

            # broadcast +-b_m*wv columns into [P, M, P] scale tiles (one-time)
            wvbf = {}
            for l in range(3):
                for sgn in ("p", "m"):
                    tl = pp.tile([P, M, P], BF16, tag=f"wvbf{sgn}{l}")
                    nc.vector.tensor_copy(
                        tl[:],
                        w[f"wvb{sgn}{l}"][:].unsqueeze(2).broadcast_to((P, M, P)))
                    wvbf[f"{sgn}{l}"] = tl

            def emit_q_side(l, xqt_t):
                """Project q and build the scaled q-features for layer l.
                Emitted before layer l-1's AllGather: no AG dependency."""
                qt_ps = ps.tile([P, P], F32, tag="qt", name=f"qt_{l}")
                for c in range(VC):
                    nc.tensor.matmul(qt_ps[:], w[f"wq{l}"][:, c, :],
                                     xqt_t[:, c, :],
                                     start=(c == 0), stop=(c == VC - 1))
                qt32 = sc.tile([P, P], F32, tag="qt32", name=f"qt32_{l}")
                nc.scalar.activation(out=qt32[:], in_=qt_ps[:], func=AF.Copy)
                thq = fq_pool.tile([P, NW, P], F32, tag="thq", name=f"thq_{l}")
                nc.vector.tensor_mul(
                    out=thq[:],
                    in0=qt32[:].unsqueeze(1).broadcast_to((P, NW, P)),
                    in1=mwt[:, :, 0:P])
                nc.vector.add_range_wrap(out=thq[:, 0:3, :], in_=thq[:, 0:3, :],
                                         shift=0.0, bound=PI, period=2 * PI)
                nc.vector.add_range_wrap(out=thq[:, 3:NW, :], in_=thq[:, 3:NW, :],
                                         shift=0.0, bound=3 * PI, period=4 * PI)
                nc.vector.add_range_wrap(out=thq[:, 3:NW, :], in_=thq[:, 3:NW, :],
                                         shift=0.0, bound=PI, period=2 * PI)
                fqp = fq_pool.tile([P, M, P], BF16, tag="fqp", name=f"fqp_{l}")
                fqm = fq_pool.tile([P, M, P], BF16, tag="fqm", name=f"fqm_{l}")
                nc.scalar.activation(out=fqp[:, 0, :], in_=qt32[:], func=AF.Sin,
                                     scale=W0, bias=biasp[:])
                nc.scalar.activation(out=fqm[:, 0, :], in_=qt32[:], func=AF.Sin,
                                     scale=W0, bias=biasm[:])
                nc.scalar.activation(out=fqp[:, 1, :], in_=qt32[:], func=AF.Sin,
                                     scale=2 * W0, bias=biasp[:])
                nc.scalar.activation(out=fqm[:, 1, :], in_=qt32[:], func=AF.Sin,
                                     scale=2 * W0, bias=biasm[:])
                nc.scalar.activation(out=fqp[:, 2:M, :], in_=thq[:], func=AF.Sin,
                                     bias=biasp[:])
                nc.scalar.activation(out=fqm[:, 2:M, :], in_=thq[:], func=AF.Sin,
                                     bias=biasm[:])
                qfp = fq_pool.tile([P, M, P], BF16, tag="qfp", name=f"qfp_{l}")
                qfm = fq_pool.tile([P, M, P], BF16, tag="qfm", name=f"qfm_{l}")
                nc.vector.tensor_mul(out=qfp[:], in0=fqp[:], in1=wvbf[f"p{l}"][:])
                nc.vector.tensor_mul(out=qfm[:], in0=fqm[:], in1=wvbf[f"m{l}"][:])
                return qfp, qfm

            qfp, qfm = emit_q_side(0, xqt)

            for l in range(3):
                # ---- k projection (needs AllGathered X^T)
                kt_ps = ps.tile([P, S], F32, tag="kt")
                for c in range(VC):
                    nc.tensor.matmul(kt_ps[:], w[f"wk{l}"][:, c, :],
                                     xft[:, c, :],
                                     start=(c == 0), stop=(c == VC - 1))
                kt32 = sc.tile([P, S], F32, tag="kt32")
                nc.scalar.activation(out=kt32[:], in_=kt_ps[:], func=AF.Copy)

                thk = fk_pool.tile([P, NW, S], F32, tag="thk", name=f"thk_{l}")
                nc.vector.tensor_mul(
                    out=thk[:],
                    in0=kt32[:].unsqueeze(1).broadcast_to((P, NW, S)),
                    in1=mwt[:])
                nc.vector.add_range_wrap(out=thk[:, 0:3, :], in_=thk[:, 0:3, :],
                                         shift=0.0, bound=PI, period=2 * PI)
                nc.vector.add_range_wrap(out=thk[:, 3:NW, :], in_=thk[:, 3:NW, :],
                                         shift=0.0, bound=3 * PI, period=4 * PI)
                nc.vector.add_range_wrap(out=thk[:, 3:NW, :], in_=thk[:, 3:NW, :],
                                         shift=0.0, bound=PI, period=2 * PI)

                fkp = fk_pool.tile([P, M, S], BF16, tag="fkp", name=f"fkp_{l}")
                fkm = fk_pool.tile([P, M, S], BF16, tag="fkm", name=f"fkm_{l}")
                nc.scalar.activation(out=fkp[:, 0, :], in_=kt32[:], func=AF.Sin,
                                     scale=W0, bias=biasp[:])
                nc.scalar.activation(out=fkm[:, 0, :], in_=kt32[:], func=AF.Sin,
                                     scale=W0, bias=biasm[:])
                nc.scalar.activation(out=fkp[:, 1, :], in_=kt32[:], func=AF.Sin,
                                     scale=2 * W0, bias=biasp[:])
                nc.scalar.activation(out=fkm[:, 1, :], in_=kt32[:], func=AF.Sin,
                                     scale=2 * W0, bias=biasm[:])
                nc.scalar.activation(out=fkp[:, 2:M, :], in_=thk[:], func=AF.Sin,
                                     bias=biasp[:])
                nc.scalar.activation(out=fkm[:, 2:M, :], in_=thk[:], func=AF.Sin,
                                     bias=biasm[:])

                # ---- scores [i, j] accumulated over 2M chunks
                sct = ps.tile([P, S], F32, tag="sct")
                for mi in range(M):
                    nc.tensor.matmul(sct[:], qfp[:, mi, :], fkp[:, mi, :],
                                     start=(mi == 0), stop=False)
                for mi in range(M):
                    nc.tensor.matmul(sct[:], qfm[:, mi, :], fkm[:, mi, :],
                                     start=False, stop=(mi == M - 1))

                # ---- softmax exp on ACT with fused row-sum
                expt = sc.tile([P, S], BF16, tag="expt")
                sums = sc.tile([P, 1], F32, tag="sums")
                nc.scalar.activation(out=expt[:], in_=sct[:], func=AF.Exp,
                                     accum_out=sums[:])
                rin = sc.tile([P, 1], F32, tag="rin")
                nc.vector.reciprocal(rin[:], sums[:])

                # ---- exp^T for attnV (PE transpose, ACT copy back)
                expT_ps = ps.tile([P, 2, P], BF16, tag="kt")
                for h in range(2):
                    nc.tensor.transpose(expT_ps[:, h, :],
                                        expt[:, h * P:(h + 1) * P], ident[:])
                expT = sc.tile([P, 2, P], BF16, tag="expT")
                nc.scalar.activation(out=expT[:], in_=expT_ps[:], func=AF.Copy)

                av = ps.tile([P, V], F32, tag="big")
                for off in range(0, V, 512):
                    n = min(512, V - off)
                    for h in range(2):
                        nc.tensor.matmul(av[:, off:off + n], expT[:, h, :],
                                         xf[:, h, off:off + n],
                                         start=(h == 0), stop=(h == 1))

                # ---- ax = av/sums + X, LN
                ax = sc.tile([P, V], F32, tag="ax")
                nc.vector.scalar_tensor_tensor(out=ax[:], in0=av[:],
                                               scalar=rin[:], in1=xo32[:],
                                               op0=ALU.mult, op1=ALU.add)
                stats = sc.tile([P, 3, 6], F32, tag="stats")
                nc.vector.bn_stats(out=stats[:, 0, :], in_=ax[:, 0:512])
                nc.vector.bn_stats(out=stats[:, 1, :], in_=ax[:, 512:1024])
                nc.vector.bn_stats(out=stats[:, 2, :], in_=ax[:, 1024:1280])
                mv = sc.tile([P, 2], F32, tag="mv")
                nc.vector.bn_aggr(out=mv[:], in_=stats[:])
                vv = sc.tile([P, 1], F32, tag="vv")
                nc.vector.tensor_scalar(out=vv[:], in0=mv[:, 1:2], scalar1=EPS,
                                        scalar2=None, op0=ALU.add)
                s_ = sc.tile([P, 1], F32, tag="s_")
                nc.vector.reciprocal(s_[:], vv[:])
                r_ = sc.tile([P, 1], F32, tag="r_")
                nc.vector.tensor_scalar(out=r_[:], in0=s_[:], scalar1=0.4315,
                                        scalar2=0.361, op0=ALU.mult, op1=ALU.add)
                t1 = sc.tile([P, 1], F32, tag="t1")
                for _ in range(2):
                    nc.vector.tensor_mul(out=t1[:], in0=vv[:], in1=r_[:])
                    nc.vector.tensor_mul(out=t1[:], in0=t1[:], in1=r_[:])
                    nc.vector.tensor_scalar(out=t1[:], in0=t1[:], scalar1=-0.5,
                                            scalar2=1.5, op0=ALU.mult, op1=ALU.add)
                    nc.vector.tensor_mul(out=r_[:], in0=r_[:], in1=t1[:])
                negms = sc.tile([P, 1], F32, tag="negms")
                nc.vector.tensor_scalar(out=negms[:], in0=mv[:, 0:1],
                                        scalar1=r_[:], scalar2=-1.0,
                                        op0=ALU.mult, op1=ALU.mult)

                # ---- LN apply on ACT (fp32 + bf16 copies)
                yb = sc.tile([P, V], BF16, tag="yb")
                nc.scalar.activation(out=yb[:], in_=ax[:], func=AF.Identity,
                                     bias=negms[:], scale=r_[:])
                y32 = sc.tile([P, V], F32, tag="y32")
                nc.scalar.activation(out=y32[:], in_=ax[:], func=AF.Identity,
                                     bias=negms[:], scale=r_[:])

                # ---- yb^T via PE (copy back on DVE)
                ybt_ps = ps2.tile([P, V], BF16, tag="tps")
                for c in range(VC):
                    nc.tensor.transpose(ybt_ps[:, c * P:(c + 1) * P],
                                        yb[:, c * P:(c + 1) * P], ident[:])
                ybt = sc.tile([P, VC, P], BF16, tag="ybt")
                nc.vector.tensor_copy(ybt[:], ybt_ps[:])

                # ---- FFN
                h1_ps = ps.tile([P, P], F32, tag="qt")
                for c in range(VC):
                    nc.tensor.matmul(h1_ps[:], w[f"w1{l}"][:, c, :],
                                     ybt[:, c, :],
                                     start=(c == 0), stop=(c == VC - 1))
                h1r = sc.tile([P, P], BF16, tag="h1r")
                nc.scalar.activation(out=h1r[:], in_=h1_ps[:], func=AF.Relu,
                                     bias=w[f"b1{l}"][:], scale=1.0)
                o2 = ps.tile([P, V], F32, tag="big")
                for off in range(0, V, 512):
                    n = min(512, V - off)
                    nc.tensor.matmul(o2[:, off:off + n], h1r[:],
                                     w[f"w2{l}"][:, off:off + n],
                                     start=True, stop=False)
                    nc.tensor.matmul(o2[:, off:off + n], onesrow[:],
                                     w[f"b2{l}"][:, off:off + n],
                                     start=False, stop=True)
                z32 = xb.tile([P, V], F32, tag="xo32")
                nc.vector.tensor_add(out=z32[:], in0=o2[:], in1=y32[:])

                if l == 2:
                    nc.sync.dma_start(zout[:, :], z32[:])
                    break

                zb = sc.tile([P, V], BF16, tag="zb")
                nc.scalar.activation(out=zb[:], in_=z32[:], func=AF.Copy)
                zbt_ps = ps2.tile([P, V], BF16, tag="tps")
                for c in range(VC):
                    nc.tensor.transpose(zbt_ps[:, c * P:(c + 1) * P],
                                        zb[:, c * P:(c + 1) * P], ident[:])
                zbt = xb.tile([P, VC, P], BF16, tag="xqt")
                nc.vector.tensor_copy(zbt[:], zbt_ps[:])

                # next layer's q side runs in the AllGather's shadow
                qfp, qfm = emit_q_side(l + 1, zbt)

                # ---- AllGather natural halves only; transpose locally
                nc.sync.dma_start(agin[l][:, :], zb[:])
                nc.gpsimd.collective_compute(
                    "AllGather", ALU.bypass, replica_groups=groups,
                    ins=[agin[l][:, :]], outs=[agout[l][:, :, :]])
                xf_n = xb.tile([P, 2, V], BF16, tag="xf")
                for r in range(2):
                    nc.sync.dma_start(xf_n[:, r, :], agout[l][r, :, :])
                xft_n = xb.tile([P, VC, S], BF16, tag="xft")
                for r in range(2):
                    tr_ps = ps2.tile([P, V], BF16, tag="tps")
                    for c in range(VC):
                        nc.tensor.transpose(tr_ps[:, c * P:(c + 1) * P],
                                            xf_n[:, r, c * P:(c + 1) * P],
                                            ident[:])
                    nc.scalar.activation(
                        out=xft_n[:, :, r * P:(r + 1) * P],
                        in_=tr_ps[:].rearrange("p (c i) -> p c i", i=P),
                        func=AF.Copy)
                xo32, xf, xft = z32, xf_n, xft_n

    nc.compile()
    return nc


def _bf(a):
    return np.ascontiguousarray(a.astype(ml_dtypes.bfloat16))


def kernel(**inputs):
    X = np.asarray(inputs["X"], dtype=np.float32)
    lys = int(np.asarray(inputs["lys_pos"]))
    if "nc" not in _CACHE:
        _CACHE["nc"] = _build()
    nc = _CACHE["nc"]

    wshared = {}
    for l, li in enumerate((1, 2, 3)):
        Wq = np.asarray(inputs[f"Wq{li}"], np.float32)
        Wk = np.asarray(inputs[f"Wk{li}"], np.float32)
        wv = np.asarray(inputs[f"wv{li}"], np.float32)
        W1 = np.asarray(inputs[f"rW1_{li}"], np.float32)
        W2 = np.asarray(inputs[f"rW2_{li}"], np.float32)
        wshared[f"wq{l}"] = _bf(Wq.reshape(VC, P, H).transpose(1, 0, 2))
        wshared[f"wk{l}"] = _bf(Wk.reshape(VC, P, H).transpose(1, 0, 2))
        wvb = wv[:, None] * np.asarray(BCO, np.float32)[None, :]
        wshared[f"wvbp{l}"] = np.ascontiguousarray(wvb.astype(np.float32))
        wshared[f"wvbm{l}"] = np.ascontiguousarray((-wvb).astype(np.float32))
        wshared[f"w1{l}"] = _bf(W1.reshape(VC, P, H).transpose(1, 0, 2))
        wshared[f"b1{l}"] = np.ascontiguousarray(
            np.asarray(inputs[f"rb1_{li}"], np.float32)[:, None])
        wshared[f"w2{l}"] = _bf(W2)
        wshared[f"b2{l}"] = _bf(
            np.asarray(inputs[f"rb2_{li}"], np.float32)[None, :])

    in_maps = []
    for c in range(NCORES):
        b, h = c // 2, c % 2
        Xb = X[b]                        # [S, V]
        Xo = Xb[h * P:(h + 1) * P]       # [P, V]
        m = dict(wshared)
        m["xo32"] = np.ascontiguousarray(Xo)
        m["xqt"] = _bf(Xo.T.reshape(VC, P, P).transpose(1, 0, 2))
        m["xf"] = _bf(Xb.reshape(2, P, V).transpose(1, 0, 2))
        m["xft"] = _bf(Xb.T.reshape(VC, P, S).transpose(1, 0, 2))
        in_maps.append(m)

    res = run_bass_kernel_spmd(nc, in_maps, core_ids=list(range(NCORES)))
    _CACHE["last_res"] = res

    X3 = np.zeros((B, S, V), np.float32)
    for c in range(NCORES):
        b, h = c // 2, c % 2
        X3[b, h * P:(h + 1) * P] = res.results[c]["zout"]

    # ---- layer 4 + head on host (fp32) ----
    def ln(x):
        m_ = x.mean(-1, keepdims=True)
        v_ = ((x - m_) ** 2).mean(-1, keepdims=True)
        return (x - m_) / np.sqrt(v_ + EPS)

    Wq4 = np.asarray(inputs["Wq4"], np.float32)
    Wk4 = np.asarray(inputs["Wk4"], np.float32)
    wv4 = np.asarray(inputs["wv4"], np.float32)
    Xl = X3[:, lys, :][:, None, :]
    q = Xl @ Wq4
    k = X3 @ Wk4
    feat = np.tanh(q[:, :, None, :] + k[:, None, :, :])
    sco = np.einsum("bijh,h->bij", feat, wv4)
    sco = sco - sco.max(-1, keepdims=True)
    a = np.exp(sco)
    a /= a.sum(-1, keepdims=True)
    att = np.einsum("bij,bjd->bid", a, X3)
    Xl = ln(att + Xl)
    h_ = np.maximum(Xl @ np.asarray(inputs["hW1"], np.float32)
                    + np.asarray(inputs["hb1"], np.float32), 0.0)
    h_ = np.maximum(h_ @ np.asarray(inputs["hW2"], np.float32)
                    + np.asarray(inputs["hb2"], np.float32), 0.0)
    logits = (h_ @ np.asarray(inputs["hW3"], np.float32)
              + np.asarray(inputs["hb3"], np.float32))[:, 0, :]
    return logits.astype(np.float32)
